# revision 2
# baseline (speedup 1.0000x reference)
"""TransformerConv 2-layer GNN encoder on 8 Trainium2 NeuronCores, v2.

Strategy (dst-sharded graph parallelism, refactored for engine balance):
  - Same node/edge binning as v1 (degree-balanced FFD: 8 cores x 20 blocks x
    128 slots, 8 edge-chunks of 128 per block).
  - Bias algebra: k-bias cancels in segment softmax; v-bias folds into the
    skip bias. So kv tables carry no bias.
  - alpha = q.k[src] + q.(ea@WeT) = q.k[src] + (q@We).ea, with Wqe = Wq^T@We
    fused on the host (weights-only preprocessing). Launch 0 emits per-node
    [k | v | q | qwe | skip]; host gathers [k_h|ea] per edge so alpha is one
    fused DVE tensor_tensor_reduce per head.
  - Value-side edge term: scatter exp-weighted raw ea (32 cols) with the same
    S matmul, multiply by We once per block after aggregation.
  - exp-weighting via DVE tensor_scalar with per-partition scalar (4x mode).
  - All engine work split across PE / ACT / DVE / GPSIMD.
"""

import sys

sys.path.insert(0, "/opt/trn_rl_repo")

import json

import numpy as np

# ----------------------------------------------------------------------------
# Problem constants (hardcoded per contract)
# ----------------------------------------------------------------------------
N, E, IN_DIM, EDGE_DIM, HID, OUT = 20000, 160000, 128, 32, 128, 64
H1 = 4
F1 = H1 * HID  # 512
NCORES = 8
BLKS = 20          # dst blocks per core
BLKN = 128         # nodes per block
NLOC = BLKS * BLKN  # 2560 nodes per core
NTOT = NCORES * NLOC  # 20480 slots
CPB = 8            # chunks per block
T = 128            # edges per chunk
ECHUNKS = BLKS * CPB  # 160 chunks per core
ELOC = ECHUNKS * T    # 20480 edge slots per core

ISQ1 = 1.0 / np.sqrt(np.float32(HID))
ISQ2 = 1.0 / np.sqrt(np.float32(OUT))
DENOM_EPS = 1e-30

# layout widths
KW = 160           # [k_h(128) | ea(32)] per head in kvkea pack
EAVW = 161         # [1 | ea(32) | v_h(128)] per head in eav pack
L0W = 2176         # l0 out: k(512) v(512) q(512) qwe(128) skip(512)
QAW = F1 + H1 * EDGE_DIM  # 640: q(512) | qwe(128)
O2W = 288          # l1 out: k2(64) v2(64) q2(64) qwe2(32) s2(64)
KW2 = OUT + EDGE_DIM   # 96
EAVW2 = 1 + EDGE_DIM + OUT  # 97
QAW2 = OUT + EDGE_DIM  # 96

# ----------------------------------------------------------------------------
# Walrus single-wait shim + NTFF profiling hook (inlined; must be
# self-contained).
# ----------------------------------------------------------------------------
_shim_installed = False


def _split_waits_in_bir(bir_bytes: bytes) -> bytes:
    d = json.loads(bir_bytes)
    for fn in d.get("functions", []):
        for blk in fn.get("blocks", []):
            new_insts = []
            for ins in blk.get("instructions", []):
                si = ins.get("sync_info") or {}
                waits = si.get("on_wait") or []
                if len(waits) > 1:
                    for k, w in enumerate(waits[:-1]):
                        ev = {
                            "name": f"{ins['name']}_wsplit{k}",
                            "opcode": "EventSemaphore",
                            "engine": ins["engine"],
                            "ins": [],
                            "outs": [],
                            "sync_info": {"on_wait": [w], "on_update": []},
                        }
                        if "debug" in ins:
                            ev["debug"] = ins["debug"]
                        new_insts.append(ev)
                    si["on_wait"] = [waits[-1]]
                new_insts.append(ins)
            blk["instructions"] = new_insts
    return json.dumps(d).encode()


def _install_shim():
    global _shim_installed
    if _shim_installed:
        return
    import concourse.bass2jax as bass2jax
    import concourse.bass_utils as bass_utils

    orig = bass_utils.compile_bir_kernel

    def wrapped(bir_json, tmpdir, neff_name="file.neff"):
        if isinstance(bir_json, str):
            bir_json = bir_json.encode()
        return orig(_split_waits_in_bir(bir_json), tmpdir, neff_name=neff_name)

    bass_utils.compile_bir_kernel = wrapped
    bass2jax.compile_bir_kernel = wrapped

    import types

    try:
        from antenv import axon_hooks  # noqa: F401
    except ImportError:
        import antenv

        mod = types.ModuleType("antenv.axon_hooks")
        _state = {"hook": None}
        mod.set_axon_ntff_profile_hook = lambda h: _state.__setitem__("hook", h)
        mod.get_axon_ntff_profile_hook = lambda: _state["hook"]
        sys.modules["antenv.axon_hooks"] = mod
        antenv.axon_hooks = mod
        try:
            from trn_agent_boot.trn_boot import _ntff_profile_via_ctypes

            hook = _ntff_profile_via_ctypes("/opt/axon/libaxon_pjrt.so")
            if hook is not None:
                mod.set_axon_ntff_profile_hook(hook)
        except Exception:
            pass
    _shim_installed = True


# ----------------------------------------------------------------------------
# Host-side graph planning (identical to v1)
# ----------------------------------------------------------------------------
class _Plan:
    pass


def _make_plan(ei: np.ndarray) -> _Plan:
    src = np.asarray(ei[0], dtype=np.int64)
    dst = np.asarray(ei[1], dtype=np.int64)
    deg = np.bincount(dst, minlength=N)

    nbins = NCORES * BLKS
    cap_e = CPB * T
    cap_n = BLKN

    order = np.argsort(-deg, kind="stable")
    bin_e = np.zeros(nbins, dtype=np.int64)
    bin_n = np.zeros(nbins, dtype=np.int64)
    node_bin = np.empty(N, dtype=np.int64)
    start = 0
    for nd in order:
        d = deg[nd]
        placed = False
        for k in range(nbins):
            b = (start + k) % nbins
            if bin_e[b] + d <= cap_e and bin_n[b] < cap_n:
                node_bin[nd] = b
                bin_e[b] += d
                bin_n[b] += 1
                start = (b + 1) % nbins
                placed = True
                break
        if not placed:
            raise RuntimeError("bin packing failed")

    node_slot = np.empty(N, dtype=np.int64)
    fill = np.zeros(nbins, dtype=np.int64)
    for nd in range(N):
        b = node_bin[nd]
        node_slot[nd] = fill[b]
        fill[b] += 1

    node_gslot = node_bin * BLKN + node_slot

    edge_bin = node_bin[dst]
    eorder = np.argsort(edge_bin, kind="stable")
    sorted_bins = edge_bin[eorder]
    bin_starts = np.searchsorted(sorted_bins, np.arange(nbins))
    bin_ends = np.searchsorted(sorted_bins, np.arange(nbins), side="right")

    edge_src_gslot = np.zeros((NCORES, ELOC), dtype=np.int64)
    edge_id = np.full((NCORES, ELOC), -1, dtype=np.int64)
    edge_dslot = np.full((NCORES, ELOC), -1, dtype=np.int64)
    for b in range(nbins):
        core = b // BLKS
        blk = b % BLKS
        s, e = bin_starts[b], bin_ends[b]
        eids = eorder[s:e]
        ne = len(eids)
        base = blk * cap_e
        edge_id[core, base : base + ne] = eids
        edge_src_gslot[core, base : base + ne] = node_gslot[src[eids]]
        edge_dslot[core, base : base + ne] = node_slot[dst[eids]]

    p = _Plan()
    p.node_gslot = node_gslot
    p.edge_src_gslot = edge_src_gslot
    p.edge_id = edge_id
    p.edge_dslot = edge_dslot
    return p


def _build_S_packed(plan):
    """S_p[core, b, t, c*BLKN + d] (scatter) and ST_p (gather), bf16."""
    import ml_dtypes

    S = np.zeros((NCORES, ECHUNKS, T, BLKN), dtype=np.float32)
    dslot = plan.edge_dslot.reshape(NCORES, ECHUNKS, T)
    c_idx, ch_idx, t_idx = np.nonzero(dslot >= 0)
    S[c_idx, ch_idx, t_idx, dslot[c_idx, ch_idx, t_idx]] = 1.0
    bf = ml_dtypes.bfloat16
    f8 = ml_dtypes.float8_e4m3fn
    S_p = np.ascontiguousarray(
        S.reshape(NCORES, BLKS, CPB, T, BLKN).transpose(0, 1, 3, 2, 4)
        .reshape(NCORES, BLKS, T, CPB * BLKN)
    ).astype(f8)
    ST_p = np.ascontiguousarray(
        S.reshape(NCORES, BLKS, CPB, T, BLKN).transpose(0, 1, 4, 2, 3)
        .reshape(NCORES, BLKS, BLKN, CPB * T)
    ).astype(f8)
    return S_p, ST_p


def _pack_edge_rows(rows):
    """[NCORES, ELOC, W] -> [NCORES, BLKS, T, CPB*W] block-chunk-major."""
    W = rows.shape[-1]
    return np.ascontiguousarray(
        rows.reshape(NCORES, BLKS, CPB, T, W).transpose(0, 1, 3, 2, 4)
        .reshape(NCORES, BLKS, T, CPB * W)
    )


def _pack_node_cols(arr):
    """[NCORES, NLOC, W] -> [NCORES, BLKN, BLKS*W] (partition-major preload)."""
    W = arr.shape[-1]
    return np.ascontiguousarray(
        arr.reshape(NCORES, BLKS, BLKN, W).transpose(0, 2, 1, 3)
        .reshape(NCORES, BLKN, BLKS * W)
    )


# ----------------------------------------------------------------------------
# Bass kernel builders
# ----------------------------------------------------------------------------
_built = {}


def _get_nc():
    import concourse.bass as bass

    return bass.Bass(target_bir_lowering=False, trn_type="TRN2")


def _build_l0():
    """Transposed node-phase layer1: out0T = (x @ [Wk|Wv|Wq|Wqe|Ws])^T.

    Weight column-chunks are the stationary operand so PE streams node
    columns continuously; biases are per-partition (ACT Identity bias AP).
    k/v chunks (no bias) evacuate on DVE, the rest on ACT.
    """
    import concourse.mybir as mybir
    from concourse.tile import TileContext

    dt = mybir.dt
    bf, f32 = dt.bfloat16, dt.float32
    nc = _get_nc()
    NCH = L0W // 128  # 17 weight chunks
    NSL = NLOC // 512  # 5 node column slices
    xT = nc.dram_tensor("xT", [IN_DIM, NLOC], bf, kind="ExternalInput")
    W0 = nc.dram_tensor("W0", [IN_DIM, L0W], bf, kind="ExternalInput")
    b0c = nc.dram_tensor("b0c", [128, NCH], f32, kind="ExternalInput")
    out0T = nc.dram_tensor("out0T", [L0W, NLOC], bf, kind="ExternalOutput")

    AF = mybir.ActivationFunctionType

    with TileContext(nc) as tc:
        with (
            tc.tile_pool(name="const", bufs=1) as cpool,
            tc.tile_pool(name="sb", bufs=4) as pool,
            tc.tile_pool(name="ps", bufs=6, space="PSUM") as psp,
        ):
            w = cpool.tile([IN_DIM, L0W], bf)
            nc.sync.dma_start(w[:], W0[:])
            bt = cpool.tile([128, NCH], f32)
            nc.sync.dma_start(bt[:], b0c[:])
            xfull = cpool.tile([IN_DIM, NLOC], bf)
            nc.sync.dma_start(xfull[:], xT[:])

            for j in range(NCH):
                for n in range(NSL):
                    ps = psp.tile([128, 512], f32, tag="ps")
                    nc.tensor.matmul(
                        ps[:], w[:, j * 128 : (j + 1) * 128],
                        xfull[:, n * 512 : (n + 1) * 512],
                        start=True, stop=True,
                    )
                    res = pool.tile([128, 512], bf, tag="res")
                    if j < 8:  # k|v chunks: no bias
                        nc.vector.tensor_copy(res[:], ps[:])
                    else:
                        nc.scalar.activation(res[:], ps[:], AF.Identity,
                                             bias=bt[:, j : j + 1])
                    nc.sync.dma_start(
                        out0T[j * 128 : (j + 1) * 128,
                              n * 512 : (n + 1) * 512], res[:])
    return nc


def _build_l1():
    """Layer-1 edge phase + fused layer-2 node phase."""
    import concourse.mybir as mybir
    from concourse.tile import TileContext

    dt = mybir.dt
    f32, bf = dt.float32, dt.bfloat16
    nc = _get_nc()

    kvkead = nc.dram_tensor("kvkea", [BLKS, T, CPB * H1 * KW], bf, kind="ExternalInput")
    eavd = nc.dram_tensor("eav", [BLKS, T, CPB * H1 * EAVW], bf, kind="ExternalInput")
    f8 = dt.float8e4
    Sd = nc.dram_tensor("S", [BLKS, T, CPB * BLKN], f8, kind="ExternalInput")
    STd = nc.dram_tensor("ST", [BLKS, BLKN, CPB * T], f8, kind="ExternalInput")
    qaugd = nc.dram_tensor("qaug", [BLKN, BLKS * QAW], bf, kind="ExternalInput")
    skipd = nc.dram_tensor("skip", [BLKN, BLKS * F1], bf, kind="ExternalInput")
    wed = nc.dram_tensor("we", [EDGE_DIM + 1, F1], bf, kind="ExternalInput")
    w2d = nc.dram_tensor("w2", [BLKN, H1 * O2W], bf, kind="ExternalInput")
    b2d = nc.dram_tensor("b2", [1, O2W], bf, kind="ExternalInput")
    onesd = nc.dram_tensor("ones", [1, BLKN], bf, kind="ExternalInput")
    identd = nc.dram_tensor("ident", [BLKN, BLKN], bf, kind="ExternalInput")
    out2 = nc.dram_tensor("out2", [NLOC, O2W], bf, kind="ExternalOutput")

    AF = mybir.ActivationFunctionType
    ALU = mybir.AluOpType

    with TileContext(nc) as tc:
        with (
            tc.tile_pool(name="const", bufs=1) as cpool,
            tc.tile_pool(name="blk", bufs=3) as bpool,
            tc.tile_pool(name="ck", bufs=6) as kpool,
            tc.tile_pool(name="ep", bufs=2) as epool,
            tc.tile_pool(name="psqt", bufs=2, space="PSUM") as psq_p,
            tc.tile_pool(name="psacc", bufs=2, space="PSUM") as psa_p,
            tc.tile_pool(name="psdea", bufs=1, space="PSUM") as psd_p,
            tc.tile_pool(name="pstail", bufs=1, space="PSUM") as pst_p,
        ):
            we = cpool.tile([EDGE_DIM + 1, F1], bf)
            nc.sync.dma_start(we[:], wed[:])
            w2 = cpool.tile([BLKN, H1 * O2W], bf)
            nc.sync.dma_start(w2[:], w2d[:])
            b2 = cpool.tile([1, O2W], bf)
            nc.sync.dma_start(b2[:], b2d[:])
            on = cpool.tile([1, BLKN], bf)
            nc.sync.dma_start(on[:], onesd[:])
            ident = cpool.tile([BLKN, BLKN], bf)
            nc.sync.dma_start(ident[:], identd[:])
            qaug_all = cpool.tile([BLKN, BLKS * QAW], bf)
            nc.sync.dma_start(qaug_all[:], qaugd[:])
            skip_all = cpool.tile([BLKN, BLKS * F1], bf)
            nc.sync.dma_start(skip_all[:], skipd[:])

            for b in range(BLKS):
                kvkea = bpool.tile([T, CPB * H1 * KW], bf, tag="kvkea")
                nc.sync.dma_start(kvkea[:], kvkead[b])
                eav = bpool.tile([T, CPB * H1 * EAVW], bf, tag="eav")
                nc.sync.dma_start(eav[:], eavd[b])
                sb_ = bpool.tile([T, CPB * BLKN], f8, tag="sb_")
                nc.sync.dma_start(sb_[:], Sd[b])
                stb = bpool.tile([BLKN, CPB * T], f8, tag="stb")
                nc.sync.dma_start(stb[:], STd[b])

                qaug = qaug_all[:, b * QAW : (b + 1) * QAW]
                skipb = skip_all[:, b * F1 : (b + 1) * F1]

                # psnum [BLKN, 512]; psdea [BLKN, 132]: [den(1)|psea(32)] x4
                psnum = psa_p.tile([BLKN, F1], f32, tag="psnum")
                psdea = psd_p.tile([BLKN, H1 * 33], f32, tag="psdea")

                def qt_mm(i):
                    """qt gather [T, 640] in one PSUM tile, halves bank-split.
                    Issued one chunk ahead so PE never stalls on the chain."""
                    st_ = stb[:, i * T : (i + 1) * T]
                    qt_ps = psq_p.tile([T, 1024], f32, tag="qt_ps")
                    nc.tensor.matmul(qt_ps[:, : 2 * KW], st_, qaug[:, : 2 * KW],
                                     start=True, stop=True)
                    nc.tensor.matmul(qt_ps[:, 512 : 512 + 2 * KW], st_,
                                     qaug[:, 2 * KW :], start=True, stop=True)
                    return qt_ps

                qt_cur = qt_mm(0)
                for i in range(CPB):
                    s_ = sb_[:, i * BLKN : (i + 1) * BLKN]

                    qts = kpool.tile([T, H1 * KW], bf, tag="qts")
                    nc.scalar.activation(
                        qts[:],
                        qt_cur[:].rearrange("p (g w) -> p g w", g=2)[:, :, : 2 * KW],
                        AF.Copy,
                    )
                    if i + 1 < CPB:
                        qt_cur = qt_mm(i + 1)

                    # alpha per head: q.k + qwe.ea over the packed 160 cols
                    kv_ = kvkea[:, i * H1 * KW : (i + 1) * H1 * KW]
                    prod = kpool.tile([T, H1 * KW], bf, tag="prod")
                    nc.vector.tensor_tensor(prod[:], qts[:], kv_, ALU.mult)
                    alpha = kpool.tile([T, H1], f32, tag="alpha")
                    nc.vector.tensor_reduce(
                        alpha[:],
                        prod[:].rearrange("p (h w) -> p h w", h=H1),
                        mybir.AxisListType.X, ALU.add,
                    )
                    exb = kpool.tile([T, H1], f32, tag="exb")
                    nc.scalar.activation(exb[:], alpha[:], AF.Exp, scale=ISQ1)

                    # exp-weighted [1|ea|v_h] per head (one broadcast mult)
                    eav_ = eav[:, i * H1 * EAVW : (i + 1) * H1 * EAVW]
                    exw = kpool.tile([T, H1 * EAVW], bf, tag="exw")
                    exbc = (
                        exb[:].rearrange("p (h o) -> p h o", h=H1)
                        .broadcast_to([T, H1, EAVW])
                    )
                    nc.gpsimd.tensor_tensor(
                        exw[:].rearrange("p (h w) -> p h w", h=H1),
                        eav_.rearrange("p (h w) -> p h w", h=H1),
                        exbc, ALU.mult,
                    )

                    # scatter: num (v part) and den|ea part
                    exw_v = exw[:].rearrange("p (h w) -> p h w", h=H1)[:, :, 33:]
                    exw_de = exw[:].rearrange("p (h w) -> p h w", h=H1)[:, :, :33]
                    nc.tensor.matmul(psnum[:], s_, exw_v,
                                     start=(i == 0), stop=False)
                    nc.tensor.matmul(psdea[:], s_, exw_de,
                                     start=(i == 0), stop=(i == CPB - 1))

                # ---- block epilogue ----
                dea_v = psdea[:].rearrange("p (h w) -> p h w", h=H1)
                den = epool.tile([BLKN, H1], f32, tag="den")
                nc.vector.tensor_scalar_max(den[:], dea_v[:, :, 0], DENOM_EPS)
                rcp = epool.tile([BLKN, H1], f32, tag="rcp")
                nc.vector.reciprocal(rcp[:], den[:])

                # fold value-side edge term: psnum_h += psea_h @ We_h^T
                # evac [den|ea-agg] whole (den row folds the v-bias via
                # we_aug's first row: psnum_h += den_h (x) bv_h + psea_h @ We_h)
                psea_sb = epool.tile([BLKN, H1 * 33], bf, tag="psea_sb")
                nc.scalar.activation(psea_sb[:], psdea[:], AF.Copy)
                pst = pst_p.tile([BLKN, F1], bf, tag="pst")
                for h in range(H1):
                    nc.tensor.transpose(
                        pst[: 33, h * BLKN : (h + 1) * BLKN],
                        psea_sb[:, h * 33 : (h + 1) * 33], ident[:],
                    )
                pseaT = epool.tile([33, H1 * BLKN], bf, tag="pseaT")
                nc.vector.tensor_copy(pseaT[:], pst[:33, :])
                for h in range(H1):
                    nc.tensor.matmul(
                        psnum[:, h * HID : (h + 1) * HID],
                        pseaT[:, h * BLKN : (h + 1) * BLKN],
                        we[:, h * HID : (h + 1) * HID],
                        start=False, stop=(h == H1 - 1),
                    )

                # h = relu(psnum*rcp + skip)
                attn = epool.tile([BLKN, F1], bf, tag="attn")
                for h in range(H1):
                    nc.scalar.activation(
                        attn[:, h * HID : (h + 1) * HID],
                        psnum[:, h * HID : (h + 1) * HID],
                        AF.Copy, scale=rcp[:, h : h + 1],
                    )
                hpre = epool.tile([BLKN, F1], bf, tag="hpre")
                nc.gpsimd.tensor_tensor(hpre[:], attn[:], skipb, ALU.add)
                hrelu = epool.tile([BLKN, F1], bf, tag="hrelu")
                nc.vector.tensor_scalar_max(hrelu[:], hpre[:], 0.0)

                # transpose h, then layer-2 node matmuls (reuses the pst
                # buffer; Tile serializes on the pseaT copy above)
                pst2 = pst_p.tile([BLKN, F1], bf, tag="pst")
                for fb in range(H1):
                    sl = slice(fb * BLKN, (fb + 1) * BLKN)
                    nc.tensor.transpose(pst2[:, sl], hrelu[:, sl], ident[:])
                hT = epool.tile([BLKN, F1], bf, tag="hT")
                nc.scalar.activation(hT[:, : 2 * BLKN], pst2[:, : 2 * BLKN], AF.Copy)
                nc.vector.tensor_copy(hT[:, 2 * BLKN :], pst2[:, 2 * BLKN :])

                # ps2 reuses the psnum tag's other buffer (freed after the
                # previous block's attn reads)
                ps2_t = psa_p.tile([BLKN, F1], f32, tag="psnum")
                ps2 = ps2_t[:, :O2W]
                for fb in range(H1):
                    nc.tensor.matmul(
                        ps2, hT[:, fb * BLKN : (fb + 1) * BLKN],
                        w2[:, fb * O2W : (fb + 1) * O2W],
                        start=(fb == 0), stop=False,
                    )
                nc.tensor.matmul(ps2, on[:], b2[:], start=False, stop=True)
                o2 = epool.tile([BLKN, O2W], bf, tag="o2")
                nc.scalar.activation(o2[:], ps2, AF.Copy)
                nc.sync.dma_start(out2[b * BLKN : (b + 1) * BLKN, :], o2[:])
    return nc


def _build_l2():
    """Layer-2 edge phase: z = attn2 + s2 (single head, C=64)."""
    import concourse.mybir as mybir
    from concourse.tile import TileContext

    dt = mybir.dt
    f32, bf = dt.float32, dt.bfloat16
    nc = _get_nc()

    kvkead = nc.dram_tensor("kvkea2", [BLKS, T, CPB * KW2], bf, kind="ExternalInput")
    eavd = nc.dram_tensor("eav2", [BLKS, T, CPB * EAVW2], bf, kind="ExternalInput")
    f8 = dt.float8e4
    Sd = nc.dram_tensor("S", [BLKS, T, CPB * BLKN], f8, kind="ExternalInput")
    STd = nc.dram_tensor("ST", [BLKS, BLKN, CPB * T], f8, kind="ExternalInput")
    qaugd = nc.dram_tensor("qaug2", [BLKN, BLKS * QAW2], bf, kind="ExternalInput")
    s2d = nc.dram_tensor("s2", [BLKN, BLKS * OUT], bf, kind="ExternalInput")
    wed = nc.dram_tensor("we2", [EDGE_DIM, OUT], bf, kind="ExternalInput")
    identd = nc.dram_tensor("ident", [BLKN, BLKN], bf, kind="ExternalInput")
    z = nc.dram_tensor("z", [NLOC, OUT], f32, kind="ExternalOutput")

    AF = mybir.ActivationFunctionType
    ALU = mybir.AluOpType

    with TileContext(nc) as tc:
        with (
            tc.tile_pool(name="const", bufs=1) as cpool,
            tc.tile_pool(name="blk", bufs=3) as bpool,
            tc.tile_pool(name="ck", bufs=6) as kpool,
            tc.tile_pool(name="ep", bufs=2) as epool,
            tc.tile_pool(name="psqt", bufs=2, space="PSUM") as psq_p,
            tc.tile_pool(name="psacc", bufs=2, space="PSUM") as psa_p,
            tc.tile_pool(name="pstail", bufs=2, space="PSUM") as pst_p,
        ):
            we = cpool.tile([EDGE_DIM, OUT], bf)
            nc.sync.dma_start(we[:], wed[:])
            ident = cpool.tile([BLKN, BLKN], bf)
            nc.sync.dma_start(ident[:], identd[:])
            qaug_all = cpool.tile([BLKN, BLKS * QAW2], bf)
            nc.sync.dma_start(qaug_all[:], qaugd[:])
            s2_all = cpool.tile([BLKN, BLKS * OUT], bf)
            nc.sync.dma_start(s2_all[:], s2d[:])

            for b in range(BLKS):
                kvkea = bpool.tile([T, CPB * KW2], bf, tag="kvkea")
                nc.sync.dma_start(kvkea[:], kvkead[b])
                eav = bpool.tile([T, CPB * EAVW2], bf, tag="eav")
                nc.sync.dma_start(eav[:], eavd[b])
                sb_ = bpool.tile([T, CPB * BLKN], f8, tag="sb_")
                nc.sync.dma_start(sb_[:], Sd[b])
                stb = bpool.tile([BLKN, CPB * T], f8, tag="stb")
                nc.sync.dma_start(stb[:], STd[b])

                qaug = qaug_all[:, b * QAW2 : (b + 1) * QAW2]
                s2b = s2_all[:, b * OUT : (b + 1) * OUT]

                psaug = psa_p.tile([BLKN, EAVW2], f32, tag="psaug")

                def qt_mm2(i):
                    st_ = stb[:, i * T : (i + 1) * T]
                    qt_ps = psq_p.tile([T, QAW2], f32, tag="qt_ps")
                    nc.tensor.matmul(qt_ps[:], st_, qaug, start=True, stop=True)
                    return qt_ps

                qt_cur = qt_mm2(0)
                for i in range(CPB):
                    s_ = sb_[:, i * BLKN : (i + 1) * BLKN]

                    qts = kpool.tile([T, QAW2], bf, tag="qts")
                    nc.scalar.activation(qts[:], qt_cur[:], AF.Copy)
                    if i + 1 < CPB:
                        qt_cur = qt_mm2(i + 1)

                    kv_ = kvkea[:, i * KW2 : (i + 1) * KW2]
                    prod = kpool.tile([T, KW2], bf, tag="prod")
                    nc.vector.tensor_tensor(prod[:], qts[:], kv_, ALU.mult)
                    alpha = kpool.tile([T, 1], f32, tag="alpha")
                    nc.vector.tensor_reduce(
                        alpha[:], prod[:], mybir.AxisListType.X, ALU.add,
                    )
                    exb = kpool.tile([T, 1], f32, tag="exb")
                    nc.scalar.activation(exb[:], alpha[:], AF.Exp, scale=ISQ2)

                    eav_ = eav[:, i * EAVW2 : (i + 1) * EAVW2]
                    exw = kpool.tile([T, EAVW2], bf, tag="exw")
                    exbc = (
                        exb[:].rearrange("p (h o) -> p h o", h=1)
                        .broadcast_to([T, 1, EAVW2])
                    )
                    nc.vector.tensor_tensor(
                        exw[:].rearrange("p (h w) -> p h w", h=1),
                        eav_.rearrange("p (h w) -> p h w", h=1),
                        exbc, ALU.mult,
                    )

                    nc.tensor.matmul(psaug[:], s_, exw[:],
                                     start=(i == 0), stop=(i == CPB - 1))

                # ---- block epilogue ----
                den = epool.tile([BLKN, 1], f32, tag="den")
                nc.vector.tensor_scalar_max(den[:], psaug[:, 0:1], DENOM_EPS)
                rcp = epool.tile([BLKN, 1], f32, tag="rcp")
                nc.vector.reciprocal(rcp[:], den[:])

                psea_sb = epool.tile([BLKN, EDGE_DIM], bf, tag="psea_sb")
                nc.scalar.activation(psea_sb[:], psaug[:, 1:33], AF.Copy)
                pst = pst_p.tile([EDGE_DIM, BLKN], bf, tag="pst")
                nc.tensor.transpose(pst[:], psea_sb[:], ident[:])
                pseaT = epool.tile([EDGE_DIM, BLKN], bf, tag="pseaT")
                nc.vector.tensor_copy(pseaT[:], pst[:])
                nc.tensor.matmul(psaug[:, 33:], pseaT[:], we[:],
                                 start=False, stop=True, skip_group_check=True)

                attn = epool.tile([BLKN, OUT], f32, tag="attn")
                nc.scalar.activation(attn[:], psaug[:, 33:], AF.Copy,
                                     scale=rcp[:])
                zb = epool.tile([BLKN, OUT], f32, tag="zb")
                nc.vector.tensor_tensor(zb[:], attn[:], s2b, ALU.add)
                nc.sync.dma_start(z[b * BLKN : (b + 1) * BLKN, :], zb[:])
    return nc


# ----------------------------------------------------------------------------
# Kernel entry point
# ----------------------------------------------------------------------------
PROFILE = False
LAST_EXEC_NS = None
LAST_TRACES = None


def kernel(**inputs):
    global LAST_EXEC_NS, LAST_TRACES
    _install_shim()
    import ml_dtypes

    from concourse import bass_utils

    bf = ml_dtypes.bfloat16

    def _run(nc, in_maps):
        r = bass_utils.run_bass_kernel_spmd(
            nc, in_maps, core_ids=list(range(NCORES)), trace=PROFILE
        )
        if PROFILE:
            _exec_ns.append(r.exec_time_ns)
            _traces.append(r.instructions_and_trace)
        return r

    _exec_ns, _traces = [], []

    x = np.asarray(inputs["x"], dtype=np.float32)
    ei = np.asarray(inputs["ei"])
    ea = np.asarray(inputs["ea"], dtype=np.float32)
    W = {k: np.asarray(v, dtype=np.float32) for k, v in inputs.items()
         if k not in ("x", "ei", "ea")}

    plan = _make_plan(ei)
    S_p, ST_p = _build_S_packed(plan)

    # gathered edge attrs [NCORES, ELOC, EDGE_DIM] (0 for pads)
    eid = plan.edge_id
    evalid = eid >= 0
    ea_g = np.zeros((NCORES, ELOC, EDGE_DIM), dtype=np.float32)
    ea_g[evalid] = ea[eid[evalid]]

    # node features in slot order
    x_slots = np.zeros((NTOT, IN_DIM), dtype=np.float32)
    x_slots[plan.node_gslot] = x
    xT_all = np.ascontiguousarray(x_slots.T).astype(bf)

    ones = np.ones((1, BLKN), dtype=np.float32).astype(bf)
    ident = np.eye(BLKN, dtype=np.float32).astype(bf)

    # ---------------- launch 0 ----------------
    # fused weights: Wqe1[h] = Wq1_h^T @ We1_h  [IN, 32]
    Wq1, We1 = W["Wq1"], W["We1"]
    Wqe1 = np.concatenate(
        [Wq1[h * HID : (h + 1) * HID].T @ We1[h * HID : (h + 1) * HID]
         for h in range(H1)], axis=1)  # [IN, 128]
    bqe1 = np.concatenate(
        [W["bq1"][h * HID : (h + 1) * HID] @ We1[h * HID : (h + 1) * HID]
         for h in range(H1)])  # [128]
    W0 = np.concatenate(
        [W["Wk1"].T, W["Wv1"].T, W["Wq1"].T, Wqe1, W["Ws1"].T], axis=1)
    bias_full = np.concatenate(
        [np.zeros(2 * F1, np.float32), W["bq1"], bqe1, W["bs1"]])
    b0c = np.ascontiguousarray(bias_full.reshape(L0W // 128, 128).T)

    if "l0" not in _built:
        _built["l0"] = _build_l0()
    in_maps0 = []
    for c in range(NCORES):
        in_maps0.append({
            "xT": np.ascontiguousarray(xT_all[:, c * NLOC : (c + 1) * NLOC]),
            "W0": W0.astype(bf),
            "b0c": b0c.astype(np.float32),
        })
    r0 = _run(_built["l0"], in_maps0)
    out0 = np.concatenate(
        [np.asarray(r0.results[c]["out0T"]).T for c in range(NCORES)], axis=0)
    k1a, v1a = out0[:, :512], out0[:, 512:1024]
    qaug1 = out0[:, 1024:1664]          # [NTOT, 640] q|qwe
    skip1 = out0[:, 1664:2176]

    # host gathers (pure data movement)
    srcs = plan.edge_src_gslot.reshape(-1)
    k_rows = k1a[srcs].reshape(NCORES, ELOC, F1)
    v_rows = v1a[srcs].reshape(NCORES, ELOC, F1)
    ea_bf = ea_g.astype(bf)

    kvkea = np.zeros((NCORES, ELOC, H1 * KW), dtype=bf)
    eav = np.zeros((NCORES, ELOC, H1 * EAVW), dtype=bf)
    for h in range(H1):
        kvkea[:, :, h * KW : h * KW + HID] = k_rows[:, :, h * HID : (h + 1) * HID]
        kvkea[:, :, h * KW + HID : (h + 1) * KW] = ea_bf
        eav[:, :, h * EAVW] = evalid.astype(bf)
        eav[:, :, h * EAVW + 1 : h * EAVW + 33] = ea_bf
        eav[:, :, h * EAVW + 33 : (h + 1) * EAVW] = \
            v_rows[:, :, h * HID : (h + 1) * HID]
    kvkea_p = _pack_edge_rows(kvkea)
    eav_p = _pack_edge_rows(eav)
    # interleave q|qwe per head to match the kvkea [k_h|ea] layout
    qaug_i = np.zeros((NTOT, QAW), dtype=bf)
    for h in range(H1):
        qaug_i[:, h * KW : h * KW + HID] = qaug1[:, h * HID : (h + 1) * HID]
        qaug_i[:, h * KW + HID : (h + 1) * KW] = \
            qaug1[:, F1 + h * EDGE_DIM : F1 + (h + 1) * EDGE_DIM]
    qaug_p = _pack_node_cols(qaug_i.reshape(NCORES, NLOC, QAW))
    skip_p = _pack_node_cols(skip1.reshape(NCORES, NLOC, F1))

    # l1 consts: W2cat [512, 288] = [Wk2^T|Wv2^T|Wq2^T|Wqe2|Ws2^T]
    Wqe2 = W["Wq2"].T @ W["We2"]  # [512, 32]
    bqe2 = W["bq2"] @ W["We2"]    # [32]
    W2cat = np.concatenate(
        [W["Wk2"].T, W["Wv2"].T, W["Wq2"].T, Wqe2, W["Ws2"].T], axis=1)
    b2row = np.concatenate(
        [np.zeros(OUT, np.float32), W["bv2"], W["bq2"], bqe2, W["bs2"]])
    w2_p = np.ascontiguousarray(
        W2cat.reshape(H1, BLKN, O2W).transpose(1, 0, 2).reshape(BLKN, -1))

    if "l1" not in _built:
        _built["l1"] = _build_l1()
    in_maps1 = []
    for c in range(NCORES):
        in_maps1.append({
            "kvkea": kvkea_p[c], "eav": eav_p[c],
            "S": S_p[c], "ST": ST_p[c],
            "qaug": qaug_p[c], "skip": skip_p[c],
            "we": np.concatenate(
                [W["bv1"][None, :], W["We1"].T], axis=0).astype(bf),
            "w2": w2_p.astype(bf),
            "b2": b2row[None, :].astype(bf),
            "ones": ones, "ident": ident,
        })
    r1 = _run(_built["l1"], in_maps1)
    out2 = np.concatenate([r1.results[c]["out2"] for c in range(NCORES)], axis=0)
    k2a, v2a = out2[:, :OUT], out2[:, OUT : 2 * OUT]
    qaug2 = out2[:, 2 * OUT : 2 * OUT + QAW2]   # q2|qwe2 (96)
    s2a = out2[:, 2 * OUT + QAW2 :]

    k2_rows = k2a[srcs].reshape(NCORES, ELOC, OUT)
    v2_rows = v2a[srcs].reshape(NCORES, ELOC, OUT)
    kvkea2 = np.zeros((NCORES, ELOC, KW2), dtype=bf)
    kvkea2[:, :, :OUT] = k2_rows
    kvkea2[:, :, OUT:] = ea_bf
    eav2 = np.zeros((NCORES, ELOC, EAVW2), dtype=bf)
    eav2[:, :, 0] = evalid.astype(bf)
    eav2[:, :, 1:33] = ea_bf
    eav2[:, :, 33:] = v2_rows
    kvkea2_p = _pack_edge_rows(kvkea2)
    eav2_p = _pack_edge_rows(eav2)
    qaug2_p = _pack_node_cols(qaug2.reshape(NCORES, NLOC, QAW2))
    s2_p = _pack_node_cols(s2a.reshape(NCORES, NLOC, OUT))

    if "l2" not in _built:
        _built["l2"] = _build_l2()
    in_maps2 = []
    for c in range(NCORES):
        in_maps2.append({
            "kvkea2": kvkea2_p[c], "eav2": eav2_p[c],
            "S": S_p[c], "ST": ST_p[c],
            "qaug2": qaug2_p[c], "s2": s2_p[c],
            "we2": np.ascontiguousarray(W["We2"].T).astype(bf),
            "ident": ident,
        })
    r2 = _run(_built["l2"], in_maps2)
    z_all = np.concatenate([r2.results[c]["z"] for c in range(NCORES)], axis=0)

    zout = z_all[plan.node_gslot]
    if PROFILE:
        LAST_EXEC_NS = sum(int(t) for t in _exec_ns if t) if all(_exec_ns) else None
        LAST_TRACES = _traces
    return zout.astype(np.float32)


# revision 3
# speedup vs baseline: 1.0039x; 1.0039x over previous
"""TransformerConv 2-layer GNN encoder on 8 Trainium2 NeuronCores, v2.

Strategy (dst-sharded graph parallelism, refactored for engine balance):
  - Same node/edge binning as v1 (degree-balanced FFD: 8 cores x 20 blocks x
    128 slots, 8 edge-chunks of 128 per block).
  - Bias algebra: k-bias cancels in segment softmax; v-bias folds into the
    skip bias. So kv tables carry no bias.
  - alpha = q.k[src] + q.(ea@WeT) = q.k[src] + (q@We).ea, with Wqe = Wq^T@We
    fused on the host (weights-only preprocessing). Launch 0 emits per-node
    [k | v | q | qwe | skip]; host gathers [k_h|ea] per edge so alpha is one
    fused DVE tensor_tensor_reduce per head.
  - Value-side edge term: scatter exp-weighted raw ea (32 cols) with the same
    S matmul, multiply by We once per block after aggregation.
  - exp-weighting via DVE tensor_scalar with per-partition scalar (4x mode).
  - All engine work split across PE / ACT / DVE / GPSIMD.
"""

import sys

sys.path.insert(0, "/opt/trn_rl_repo")

import json

import numpy as np

# ----------------------------------------------------------------------------
# Problem constants (hardcoded per contract)
# ----------------------------------------------------------------------------
N, E, IN_DIM, EDGE_DIM, HID, OUT = 20000, 160000, 128, 32, 128, 64
H1 = 4
F1 = H1 * HID  # 512
NCORES = 8
BLKS = 20          # dst blocks per core
BLKN = 128         # nodes per block
NLOC = BLKS * BLKN  # 2560 nodes per core
NTOT = NCORES * NLOC  # 20480 slots
CPB = 8            # chunks per block
T = 128            # edges per chunk
ECHUNKS = BLKS * CPB  # 160 chunks per core
ELOC = ECHUNKS * T    # 20480 edge slots per core

ISQ1 = 1.0 / np.sqrt(np.float32(HID))
ISQ2 = 1.0 / np.sqrt(np.float32(OUT))
DENOM_EPS = 1e-30

# layout widths
KW = 160           # [k_h(128) | ea(32)] per head in kvkea pack
EAVW = 161         # [1 | ea(32) | v_h(128)] per head in eav pack
L0W = 2176         # l0 out: k(512) v(512) q(512) qwe(128) skip(512)
QAW = F1 + H1 * EDGE_DIM  # 640: q(512) | qwe(128)
O2W = 288          # l1 out: k2(64) v2(64) q2(64) qwe2(32) s2(64)
KW2 = OUT + EDGE_DIM   # 96
EAVW2 = 1 + EDGE_DIM + OUT  # 97
QAW2 = OUT + EDGE_DIM  # 96

# ----------------------------------------------------------------------------
# Walrus single-wait shim + NTFF profiling hook (inlined; must be
# self-contained).
# ----------------------------------------------------------------------------
_shim_installed = False


def _split_waits_in_bir(bir_bytes: bytes) -> bytes:
    d = json.loads(bir_bytes)
    for fn in d.get("functions", []):
        for blk in fn.get("blocks", []):
            new_insts = []
            for ins in blk.get("instructions", []):
                si = ins.get("sync_info") or {}
                waits = si.get("on_wait") or []
                if len(waits) > 1:
                    for k, w in enumerate(waits[:-1]):
                        ev = {
                            "name": f"{ins['name']}_wsplit{k}",
                            "opcode": "EventSemaphore",
                            "engine": ins["engine"],
                            "ins": [],
                            "outs": [],
                            "sync_info": {"on_wait": [w], "on_update": []},
                        }
                        if "debug" in ins:
                            ev["debug"] = ins["debug"]
                        new_insts.append(ev)
                    si["on_wait"] = [waits[-1]]
                new_insts.append(ins)
            blk["instructions"] = new_insts
    return json.dumps(d).encode()


def _install_shim():
    global _shim_installed
    if _shim_installed:
        return
    import concourse.bass2jax as bass2jax
    import concourse.bass_utils as bass_utils

    orig = bass_utils.compile_bir_kernel

    def wrapped(bir_json, tmpdir, neff_name="file.neff"):
        if isinstance(bir_json, str):
            bir_json = bir_json.encode()
        return orig(_split_waits_in_bir(bir_json), tmpdir, neff_name=neff_name)

    bass_utils.compile_bir_kernel = wrapped
    bass2jax.compile_bir_kernel = wrapped

    import types

    try:
        from antenv import axon_hooks  # noqa: F401
    except ImportError:
        import antenv

        mod = types.ModuleType("antenv.axon_hooks")
        _state = {"hook": None}
        mod.set_axon_ntff_profile_hook = lambda h: _state.__setitem__("hook", h)
        mod.get_axon_ntff_profile_hook = lambda: _state["hook"]
        sys.modules["antenv.axon_hooks"] = mod
        antenv.axon_hooks = mod
        try:
            from trn_agent_boot.trn_boot import _ntff_profile_via_ctypes

            hook = _ntff_profile_via_ctypes("/opt/axon/libaxon_pjrt.so")
            if hook is not None:
                mod.set_axon_ntff_profile_hook(hook)
        except Exception:
            pass
    _shim_installed = True


# ----------------------------------------------------------------------------
# Host-side graph planning (identical to v1)
# ----------------------------------------------------------------------------
class _Plan:
    pass


def _make_plan(ei: np.ndarray) -> _Plan:
    src = np.asarray(ei[0], dtype=np.int64)
    dst = np.asarray(ei[1], dtype=np.int64)
    deg = np.bincount(dst, minlength=N)

    nbins = NCORES * BLKS
    cap_e = CPB * T
    cap_n = BLKN

    order = np.argsort(-deg, kind="stable")
    bin_e = np.zeros(nbins, dtype=np.int64)
    bin_n = np.zeros(nbins, dtype=np.int64)
    node_bin = np.empty(N, dtype=np.int64)
    start = 0
    for nd in order:
        d = deg[nd]
        placed = False
        for k in range(nbins):
            b = (start + k) % nbins
            if bin_e[b] + d <= cap_e and bin_n[b] < cap_n:
                node_bin[nd] = b
                bin_e[b] += d
                bin_n[b] += 1
                start = (b + 1) % nbins
                placed = True
                break
        if not placed:
            raise RuntimeError("bin packing failed")

    node_slot = np.empty(N, dtype=np.int64)
    fill = np.zeros(nbins, dtype=np.int64)
    for nd in range(N):
        b = node_bin[nd]
        node_slot[nd] = fill[b]
        fill[b] += 1

    node_gslot = node_bin * BLKN + node_slot

    edge_bin = node_bin[dst]
    eorder = np.argsort(edge_bin, kind="stable")
    sorted_bins = edge_bin[eorder]
    bin_starts = np.searchsorted(sorted_bins, np.arange(nbins))
    bin_ends = np.searchsorted(sorted_bins, np.arange(nbins), side="right")

    edge_src_gslot = np.zeros((NCORES, ELOC), dtype=np.int64)
    edge_id = np.full((NCORES, ELOC), -1, dtype=np.int64)
    edge_dslot = np.full((NCORES, ELOC), -1, dtype=np.int64)
    for b in range(nbins):
        core = b // BLKS
        blk = b % BLKS
        s, e = bin_starts[b], bin_ends[b]
        eids = eorder[s:e]
        ne = len(eids)
        base = blk * cap_e
        edge_id[core, base : base + ne] = eids
        edge_src_gslot[core, base : base + ne] = node_gslot[src[eids]]
        edge_dslot[core, base : base + ne] = node_slot[dst[eids]]

    p = _Plan()
    p.node_gslot = node_gslot
    p.edge_src_gslot = edge_src_gslot
    p.edge_id = edge_id
    p.edge_dslot = edge_dslot
    return p


def _build_S_packed(plan):
    """S_p[core, b, t, c*BLKN + d] (scatter) and ST_p (gather), bf16."""
    import ml_dtypes

    S = np.zeros((NCORES, ECHUNKS, T, BLKN), dtype=np.float32)
    dslot = plan.edge_dslot.reshape(NCORES, ECHUNKS, T)
    c_idx, ch_idx, t_idx = np.nonzero(dslot >= 0)
    S[c_idx, ch_idx, t_idx, dslot[c_idx, ch_idx, t_idx]] = 1.0
    bf = ml_dtypes.bfloat16
    f8 = ml_dtypes.float8_e4m3fn
    S_p = np.ascontiguousarray(
        S.reshape(NCORES, BLKS, CPB, T, BLKN).transpose(0, 1, 3, 2, 4)
        .reshape(NCORES, BLKS, T, CPB * BLKN)
    ).astype(f8)
    ST_p = np.ascontiguousarray(
        S.reshape(NCORES, BLKS, CPB, T, BLKN).transpose(0, 1, 4, 2, 3)
        .reshape(NCORES, BLKS, BLKN, CPB * T)
    ).astype(f8)
    return S_p, ST_p


def _pack_edge_rows(rows):
    """[NCORES, ELOC, W] -> [NCORES, BLKS, T, CPB*W] block-chunk-major."""
    W = rows.shape[-1]
    return np.ascontiguousarray(
        rows.reshape(NCORES, BLKS, CPB, T, W).transpose(0, 1, 3, 2, 4)
        .reshape(NCORES, BLKS, T, CPB * W)
    )


def _pack_node_cols(arr):
    """[NCORES, NLOC, W] -> [NCORES, BLKN, BLKS*W] (partition-major preload)."""
    W = arr.shape[-1]
    return np.ascontiguousarray(
        arr.reshape(NCORES, BLKS, BLKN, W).transpose(0, 2, 1, 3)
        .reshape(NCORES, BLKN, BLKS * W)
    )


# ----------------------------------------------------------------------------
# Bass kernel builders
# ----------------------------------------------------------------------------
_built = {}


def _get_nc():
    import concourse.bass as bass

    return bass.Bass(target_bir_lowering=False, trn_type="TRN2")


def _build_l0():
    """Transposed node-phase layer1: out0T = (x @ [Wk|Wv|Wq|Wqe|Ws])^T.

    Weight column-chunks are the stationary operand so PE streams node
    columns continuously; biases are per-partition (ACT Identity bias AP).
    k/v chunks (no bias) evacuate on DVE, the rest on ACT.
    """
    import concourse.mybir as mybir
    from concourse.tile import TileContext

    dt = mybir.dt
    bf, f32 = dt.bfloat16, dt.float32
    nc = _get_nc()
    NCH = L0W // 128  # 17 weight chunks
    NSL = NLOC // 512  # 5 node column slices
    xT = nc.dram_tensor("xT", [IN_DIM, NLOC], bf, kind="ExternalInput")
    W0 = nc.dram_tensor("W0", [IN_DIM, L0W], bf, kind="ExternalInput")
    b0c = nc.dram_tensor("b0c", [128, NCH], f32, kind="ExternalInput")
    out0T = nc.dram_tensor("out0T", [L0W, NLOC], bf, kind="ExternalOutput")

    AF = mybir.ActivationFunctionType

    with TileContext(nc) as tc:
        with (
            tc.tile_pool(name="const", bufs=1) as cpool,
            tc.tile_pool(name="sb", bufs=4) as pool,
            tc.tile_pool(name="ps", bufs=6, space="PSUM") as psp,
        ):
            w = cpool.tile([IN_DIM, L0W], bf)
            nc.sync.dma_start(w[:], W0[:])
            bt = cpool.tile([128, NCH], f32)
            nc.sync.dma_start(bt[:], b0c[:])
            xfull = cpool.tile([IN_DIM, NLOC], bf)
            nc.sync.dma_start(xfull[:], xT[:])

            for j in range(NCH):
                for n in range(NSL):
                    ps = psp.tile([128, 512], f32, tag="ps")
                    nc.tensor.matmul(
                        ps[:], w[:, j * 128 : (j + 1) * 128],
                        xfull[:, n * 512 : (n + 1) * 512],
                        start=True, stop=True,
                    )
                    res = pool.tile([128, 512], bf, tag="res")
                    if j < 8:  # k|v chunks: no bias
                        nc.vector.tensor_copy(res[:], ps[:])
                    else:
                        nc.scalar.activation(res[:], ps[:], AF.Identity,
                                             bias=bt[:, j : j + 1])
                    nc.sync.dma_start(
                        out0T[j * 128 : (j + 1) * 128,
                              n * 512 : (n + 1) * 512], res[:])
    return nc


def _build_l1():
    """Layer-1 edge phase + fused layer-2 node phase."""
    import concourse.mybir as mybir
    from concourse.tile import TileContext

    dt = mybir.dt
    f32, bf = dt.float32, dt.bfloat16
    nc = _get_nc()

    kvkead = nc.dram_tensor("kvkea", [BLKS, T, CPB * H1 * KW], bf, kind="ExternalInput")
    eavd = nc.dram_tensor("eav", [BLKS, T, CPB * H1 * EAVW], bf, kind="ExternalInput")
    f8 = dt.float8e4
    Sd = nc.dram_tensor("S", [BLKS, T, CPB * BLKN], f8, kind="ExternalInput")
    STd = nc.dram_tensor("ST", [BLKS, BLKN, CPB * T], f8, kind="ExternalInput")
    qaugd = nc.dram_tensor("qaug", [BLKN, BLKS * QAW], bf, kind="ExternalInput")
    skipd = nc.dram_tensor("skip", [BLKN, BLKS * F1], bf, kind="ExternalInput")
    wed = nc.dram_tensor("we", [EDGE_DIM + 1, F1], bf, kind="ExternalInput")
    w2d = nc.dram_tensor("w2", [BLKN, H1 * O2W], bf, kind="ExternalInput")
    b2d = nc.dram_tensor("b2", [1, O2W], bf, kind="ExternalInput")
    onesd = nc.dram_tensor("ones", [1, BLKN], bf, kind="ExternalInput")
    identd = nc.dram_tensor("ident", [BLKN, BLKN], bf, kind="ExternalInput")
    out2 = nc.dram_tensor("out2", [NLOC, O2W], bf, kind="ExternalOutput")

    AF = mybir.ActivationFunctionType
    ALU = mybir.AluOpType

    with TileContext(nc) as tc:
        with (
            tc.tile_pool(name="const", bufs=1) as cpool,
            tc.tile_pool(name="blk", bufs=3) as bpool,
            tc.tile_pool(name="ck", bufs=6) as kpool,
            tc.tile_pool(name="ep", bufs=2) as epool,
            tc.tile_pool(name="psqt", bufs=2, space="PSUM") as psq_p,
            tc.tile_pool(name="psacc", bufs=2, space="PSUM") as psa_p,
            tc.tile_pool(name="psdea", bufs=1, space="PSUM") as psd_p,
            tc.tile_pool(name="pstail", bufs=1, space="PSUM") as pst_p,
        ):
            we = cpool.tile([EDGE_DIM + 1, F1], bf)
            nc.sync.dma_start(we[:], wed[:])
            w2 = cpool.tile([BLKN, H1 * O2W], bf)
            nc.sync.dma_start(w2[:], w2d[:])
            b2 = cpool.tile([1, O2W], bf)
            nc.sync.dma_start(b2[:], b2d[:])
            on = cpool.tile([1, BLKN], bf)
            nc.sync.dma_start(on[:], onesd[:])
            ident = cpool.tile([BLKN, BLKN], bf)
            nc.sync.dma_start(ident[:], identd[:])
            qaug_all = cpool.tile([BLKN, BLKS * QAW], bf)
            nc.sync.dma_start(qaug_all[:], qaugd[:])
            skip_all = cpool.tile([BLKN, BLKS * F1], bf)
            nc.sync.dma_start(skip_all[:], skipd[:])

            for b in range(BLKS):
                kvkea = bpool.tile([T, CPB * H1 * KW], bf, tag="kvkea")
                nc.sync.dma_start(kvkea[:], kvkead[b])
                eav = bpool.tile([T, CPB * H1 * EAVW], bf, tag="eav")
                nc.sync.dma_start(eav[:], eavd[b])
                sb_ = bpool.tile([T, CPB * BLKN], f8, tag="sb_")
                nc.sync.dma_start(sb_[:], Sd[b])
                stb = bpool.tile([BLKN, CPB * T], f8, tag="stb")
                nc.sync.dma_start(stb[:], STd[b])

                qaug = qaug_all[:, b * QAW : (b + 1) * QAW]
                skipb = skip_all[:, b * F1 : (b + 1) * F1]

                # psnum [BLKN, 512]; psdea [BLKN, 132]: [den(1)|psea(32)] x4
                psnum = psa_p.tile([BLKN, F1], f32, tag="psnum")
                psdea = psd_p.tile([BLKN, H1 * 33], f32, tag="psdea")

                def qt_mm(i):
                    """qt gather [T, 640] in one PSUM tile, halves bank-split.
                    Issued one chunk ahead so PE never stalls on the chain."""
                    st_ = stb[:, i * T : (i + 1) * T]
                    qt_ps = psq_p.tile([T, 1024], f32, tag="qt_ps")
                    nc.tensor.matmul(qt_ps[:, : 2 * KW], st_, qaug[:, : 2 * KW],
                                     start=True, stop=True)
                    nc.tensor.matmul(qt_ps[:, 512 : 512 + 2 * KW], st_,
                                     qaug[:, 2 * KW :], start=True, stop=True)
                    return qt_ps

                qt_cur = qt_mm(0)
                for i in range(CPB):
                    s_ = sb_[:, i * BLKN : (i + 1) * BLKN]

                    qts = kpool.tile([T, H1 * KW], bf, tag="qts")
                    nc.scalar.activation(
                        qts[:],
                        qt_cur[:].rearrange("p (g w) -> p g w", g=2)[:, :, : 2 * KW],
                        AF.Copy,
                    )
                    if i + 1 < CPB:
                        qt_cur = qt_mm(i + 1)

                    # alpha per head: q.k + qwe.ea over the packed 160 cols
                    kv_ = kvkea[:, i * H1 * KW : (i + 1) * H1 * KW]
                    prod = kpool.tile([T, H1 * KW], bf, tag="prod")
                    nc.vector.tensor_tensor(prod[:], qts[:], kv_, ALU.mult)
                    alpha = kpool.tile([T, H1], f32, tag="alpha")
                    nc.vector.tensor_reduce(
                        alpha[:],
                        prod[:].rearrange("p (h w) -> p h w", h=H1),
                        mybir.AxisListType.X, ALU.add,
                    )
                    exb = kpool.tile([T, H1], f32, tag="exb")
                    nc.scalar.activation(exb[:], alpha[:], AF.Exp, scale=ISQ1)

                    # exp-weighted [1|ea|v_h] per head (one broadcast mult)
                    eav_ = eav[:, i * H1 * EAVW : (i + 1) * H1 * EAVW]
                    exw = kpool.tile([T, H1 * EAVW], bf, tag="exw")
                    exbc = (
                        exb[:].rearrange("p (h o) -> p h o", h=H1)
                        .broadcast_to([T, H1, EAVW])
                    )
                    nc.gpsimd.tensor_tensor(
                        exw[:].rearrange("p (h w) -> p h w", h=H1),
                        eav_.rearrange("p (h w) -> p h w", h=H1),
                        exbc, ALU.mult,
                    )

                    # scatter: num (v part) and den|ea part
                    exw_v = exw[:].rearrange("p (h w) -> p h w", h=H1)[:, :, 33:]
                    exw_de = exw[:].rearrange("p (h w) -> p h w", h=H1)[:, :, :33]
                    nc.tensor.matmul(psnum[:], s_, exw_v,
                                     start=(i == 0), stop=False)
                    nc.tensor.matmul(psdea[:], s_, exw_de,
                                     start=(i == 0), stop=(i == CPB - 1))

                # ---- block epilogue ----
                dea_v = psdea[:].rearrange("p (h w) -> p h w", h=H1)
                den = epool.tile([BLKN, H1], f32, tag="den")
                nc.vector.tensor_scalar_max(den[:], dea_v[:, :, 0], DENOM_EPS)
                rcp = epool.tile([BLKN, H1], f32, tag="rcp")
                nc.vector.reciprocal(rcp[:], den[:])

                # fold value-side edge term: psnum_h += psea_h @ We_h^T
                # evac [den|ea-agg] whole (den row folds the v-bias via
                # we_aug's first row: psnum_h += den_h (x) bv_h + psea_h @ We_h)
                psea_sb = epool.tile([BLKN, H1 * 33], bf, tag="psea_sb")
                nc.scalar.activation(psea_sb[:], psdea[:], AF.Copy)
                pst = pst_p.tile([BLKN, F1], bf, tag="pst")
                for h in range(H1):
                    nc.tensor.transpose(
                        pst[: 33, h * BLKN : (h + 1) * BLKN],
                        psea_sb[:, h * 33 : (h + 1) * 33], ident[:],
                    )
                pseaT = epool.tile([33, H1 * BLKN], bf, tag="pseaT")
                nc.vector.tensor_copy(pseaT[:], pst[:33, :])
                for h in range(H1):
                    nc.tensor.matmul(
                        psnum[:, h * HID : (h + 1) * HID],
                        pseaT[:, h * BLKN : (h + 1) * BLKN],
                        we[:, h * HID : (h + 1) * HID],
                        start=False, stop=(h == H1 - 1),
                    )

                # h = relu(psnum*rcp + skip)
                attn = epool.tile([BLKN, F1], bf, tag="attn")
                for h in range(H1):
                    nc.scalar.activation(
                        attn[:, h * HID : (h + 1) * HID],
                        psnum[:, h * HID : (h + 1) * HID],
                        AF.Copy, scale=rcp[:, h : h + 1],
                    )
                hpre = epool.tile([BLKN, F1], bf, tag="hpre")
                nc.gpsimd.tensor_tensor(hpre[:], attn[:], skipb, ALU.add)
                hrelu = epool.tile([BLKN, F1], bf, tag="hrelu")
                nc.vector.tensor_scalar_max(hrelu[:], hpre[:], 0.0)

                # transpose h, then layer-2 node matmuls (reuses the pst
                # buffer; Tile serializes on the pseaT copy above)
                pst2 = pst_p.tile([BLKN, F1], bf, tag="pst")
                for fb in range(H1):
                    sl = slice(fb * BLKN, (fb + 1) * BLKN)
                    nc.tensor.transpose(pst2[:, sl], hrelu[:, sl], ident[:])
                hT = epool.tile([BLKN, F1], bf, tag="hT")
                nc.scalar.activation(hT[:, : 2 * BLKN], pst2[:, : 2 * BLKN], AF.Copy)
                nc.vector.tensor_copy(hT[:, 2 * BLKN :], pst2[:, 2 * BLKN :])

                # ps2 reuses the psnum tag's other buffer (freed after the
                # previous block's attn reads)
                ps2_t = psa_p.tile([BLKN, F1], f32, tag="psnum")
                ps2 = ps2_t[:, :O2W]
                for fb in range(H1):
                    nc.tensor.matmul(
                        ps2, hT[:, fb * BLKN : (fb + 1) * BLKN],
                        w2[:, fb * O2W : (fb + 1) * O2W],
                        start=(fb == 0), stop=False,
                    )
                nc.tensor.matmul(ps2, on[:], b2[:], start=False, stop=True)
                o2 = epool.tile([BLKN, O2W], bf, tag="o2")
                nc.scalar.activation(o2[:], ps2, AF.Copy)
                nc.sync.dma_start(out2[b * BLKN : (b + 1) * BLKN, :], o2[:])
    return nc


def _build_l2():
    """Layer-2 edge phase: z = attn2 + s2 (single head, C=64)."""
    import concourse.mybir as mybir
    from concourse.tile import TileContext

    dt = mybir.dt
    f32, bf = dt.float32, dt.bfloat16
    nc = _get_nc()

    kvkead = nc.dram_tensor("kvkea2", [BLKS, T, CPB * KW2], bf, kind="ExternalInput")
    eavd = nc.dram_tensor("eav2", [BLKS, T, CPB * EAVW2], bf, kind="ExternalInput")
    f8 = dt.float8e4
    Sd = nc.dram_tensor("S", [BLKS, T, CPB * BLKN], f8, kind="ExternalInput")
    STd = nc.dram_tensor("ST", [BLKS, BLKN, CPB * T], f8, kind="ExternalInput")
    qaugd = nc.dram_tensor("qaug2", [BLKN, BLKS * QAW2], bf, kind="ExternalInput")
    s2d = nc.dram_tensor("s2", [BLKN, BLKS * OUT], bf, kind="ExternalInput")
    wed = nc.dram_tensor("we2", [EDGE_DIM, OUT], bf, kind="ExternalInput")
    identd = nc.dram_tensor("ident", [BLKN, BLKN], bf, kind="ExternalInput")
    z = nc.dram_tensor("z", [NLOC, OUT], f32, kind="ExternalOutput")

    AF = mybir.ActivationFunctionType
    ALU = mybir.AluOpType

    with TileContext(nc) as tc:
        with (
            tc.tile_pool(name="const", bufs=1) as cpool,
            tc.tile_pool(name="blk", bufs=3) as bpool,
            tc.tile_pool(name="ck", bufs=6) as kpool,
            tc.tile_pool(name="ep", bufs=2) as epool,
            tc.tile_pool(name="psqt", bufs=2, space="PSUM") as psq_p,
            tc.tile_pool(name="psacc", bufs=2, space="PSUM") as psa_p,
            tc.tile_pool(name="pstail", bufs=2, space="PSUM") as pst_p,
        ):
            we = cpool.tile([EDGE_DIM, OUT], bf)
            nc.sync.dma_start(we[:], wed[:])
            ident = cpool.tile([BLKN, BLKN], bf)
            nc.sync.dma_start(ident[:], identd[:])
            qaug_all = cpool.tile([BLKN, BLKS * QAW2], bf)
            nc.sync.dma_start(qaug_all[:], qaugd[:])
            s2_all = cpool.tile([BLKN, BLKS * OUT], bf)
            nc.sync.dma_start(s2_all[:], s2d[:])

            for b in range(BLKS):
                kvkea = bpool.tile([T, CPB * KW2], bf, tag="kvkea")
                nc.sync.dma_start(kvkea[:], kvkead[b])
                eav = bpool.tile([T, CPB * EAVW2], bf, tag="eav")
                nc.sync.dma_start(eav[:], eavd[b])
                sb_ = bpool.tile([T, CPB * BLKN], f8, tag="sb_")
                nc.sync.dma_start(sb_[:], Sd[b])
                stb = bpool.tile([BLKN, CPB * T], f8, tag="stb")
                nc.sync.dma_start(stb[:], STd[b])

                qaug = qaug_all[:, b * QAW2 : (b + 1) * QAW2]
                s2b = s2_all[:, b * OUT : (b + 1) * OUT]

                psaug = psa_p.tile([BLKN, EAVW2], f32, tag="psaug")

                def qt_mm2(i):
                    st_ = stb[:, i * T : (i + 1) * T]
                    qt_ps = psq_p.tile([T, QAW2], f32, tag="qt_ps")
                    nc.tensor.matmul(qt_ps[:], st_, qaug, start=True, stop=True)
                    return qt_ps

                qt_cur = qt_mm2(0)
                for i in range(CPB):
                    s_ = sb_[:, i * BLKN : (i + 1) * BLKN]

                    qts = kpool.tile([T, QAW2], bf, tag="qts")
                    nc.scalar.activation(qts[:], qt_cur[:], AF.Copy)
                    if i + 1 < CPB:
                        qt_cur = qt_mm2(i + 1)

                    kv_ = kvkea[:, i * KW2 : (i + 1) * KW2]
                    prod = kpool.tile([T, KW2], bf, tag="prod")
                    nc.vector.tensor_tensor(prod[:], qts[:], kv_, ALU.mult)
                    alpha = kpool.tile([T, 1], f32, tag="alpha")
                    nc.vector.tensor_reduce(
                        alpha[:], prod[:], mybir.AxisListType.X, ALU.add,
                    )
                    exb = kpool.tile([T, 1], f32, tag="exb")
                    nc.scalar.activation(exb[:], alpha[:], AF.Exp, scale=ISQ2)

                    eav_ = eav[:, i * EAVW2 : (i + 1) * EAVW2]
                    exw = kpool.tile([T, EAVW2], bf, tag="exw")
                    exbc = (
                        exb[:].rearrange("p (h o) -> p h o", h=1)
                        .broadcast_to([T, 1, EAVW2])
                    )
                    nc.gpsimd.tensor_tensor(
                        exw[:].rearrange("p (h w) -> p h w", h=1),
                        eav_.rearrange("p (h w) -> p h w", h=1),
                        exbc, ALU.mult,
                    )

                    nc.tensor.matmul(psaug[:], s_, exw[:],
                                     start=(i == 0), stop=(i == CPB - 1))

                # ---- block epilogue ----
                den = epool.tile([BLKN, 1], f32, tag="den")
                nc.vector.tensor_scalar_max(den[:], psaug[:, 0:1], DENOM_EPS)
                rcp = epool.tile([BLKN, 1], f32, tag="rcp")
                nc.vector.reciprocal(rcp[:], den[:])

                psea_sb = epool.tile([BLKN, EDGE_DIM], bf, tag="psea_sb")
                nc.scalar.activation(psea_sb[:], psaug[:, 1:33], AF.Copy)
                pst = pst_p.tile([EDGE_DIM, BLKN], bf, tag="pst")
                nc.tensor.transpose(pst[:], psea_sb[:], ident[:])
                pseaT = epool.tile([EDGE_DIM, BLKN], bf, tag="pseaT")
                nc.vector.tensor_copy(pseaT[:], pst[:])
                nc.tensor.matmul(psaug[:, 33:], pseaT[:], we[:],
                                 start=False, stop=True, skip_group_check=True)

                attn = epool.tile([BLKN, OUT], f32, tag="attn")
                nc.scalar.activation(attn[:], psaug[:, 33:], AF.Copy,
                                     scale=rcp[:])
                zb = epool.tile([BLKN, OUT], f32, tag="zb")
                nc.gpsimd.tensor_tensor(zb[:], attn[:], s2b, ALU.add)
                nc.sync.dma_start(z[b * BLKN : (b + 1) * BLKN, :], zb[:])
    return nc


# ----------------------------------------------------------------------------
# Kernel entry point
# ----------------------------------------------------------------------------
PROFILE = False
LAST_EXEC_NS = None
LAST_TRACES = None


def kernel(**inputs):
    global LAST_EXEC_NS, LAST_TRACES
    _install_shim()
    import ml_dtypes

    from concourse import bass_utils

    bf = ml_dtypes.bfloat16

    def _run(nc, in_maps):
        r = bass_utils.run_bass_kernel_spmd(
            nc, in_maps, core_ids=list(range(NCORES)), trace=PROFILE
        )
        if PROFILE:
            _exec_ns.append(r.exec_time_ns)
            _traces.append(r.instructions_and_trace)
        return r

    _exec_ns, _traces = [], []

    x = np.asarray(inputs["x"], dtype=np.float32)
    ei = np.asarray(inputs["ei"])
    ea = np.asarray(inputs["ea"], dtype=np.float32)
    W = {k: np.asarray(v, dtype=np.float32) for k, v in inputs.items()
         if k not in ("x", "ei", "ea")}

    plan = _make_plan(ei)
    S_p, ST_p = _build_S_packed(plan)

    # gathered edge attrs [NCORES, ELOC, EDGE_DIM] (0 for pads)
    eid = plan.edge_id
    evalid = eid >= 0
    ea_g = np.zeros((NCORES, ELOC, EDGE_DIM), dtype=np.float32)
    ea_g[evalid] = ea[eid[evalid]]

    # node features in slot order
    x_slots = np.zeros((NTOT, IN_DIM), dtype=np.float32)
    x_slots[plan.node_gslot] = x
    xT_all = np.ascontiguousarray(x_slots.T).astype(bf)

    ones = np.ones((1, BLKN), dtype=np.float32).astype(bf)
    ident = np.eye(BLKN, dtype=np.float32).astype(bf)

    # ---------------- launch 0 ----------------
    # fused weights: Wqe1[h] = Wq1_h^T @ We1_h  [IN, 32]
    Wq1, We1 = W["Wq1"], W["We1"]
    Wqe1 = np.concatenate(
        [Wq1[h * HID : (h + 1) * HID].T @ We1[h * HID : (h + 1) * HID]
         for h in range(H1)], axis=1)  # [IN, 128]
    bqe1 = np.concatenate(
        [W["bq1"][h * HID : (h + 1) * HID] @ We1[h * HID : (h + 1) * HID]
         for h in range(H1)])  # [128]
    W0 = np.concatenate(
        [W["Wk1"].T, W["Wv1"].T, W["Wq1"].T, Wqe1, W["Ws1"].T], axis=1)
    bias_full = np.concatenate(
        [np.zeros(2 * F1, np.float32), W["bq1"], bqe1, W["bs1"]])
    b0c = np.ascontiguousarray(bias_full.reshape(L0W // 128, 128).T)

    if "l0" not in _built:
        _built["l0"] = _build_l0()
    in_maps0 = []
    for c in range(NCORES):
        in_maps0.append({
            "xT": np.ascontiguousarray(xT_all[:, c * NLOC : (c + 1) * NLOC]),
            "W0": W0.astype(bf),
            "b0c": b0c.astype(np.float32),
        })
    r0 = _run(_built["l0"], in_maps0)
    out0 = np.concatenate(
        [np.asarray(r0.results[c]["out0T"]).T for c in range(NCORES)], axis=0)
    k1a, v1a = out0[:, :512], out0[:, 512:1024]
    qaug1 = out0[:, 1024:1664]          # [NTOT, 640] q|qwe
    skip1 = out0[:, 1664:2176]

    # host gathers (pure data movement)
    srcs = plan.edge_src_gslot.reshape(-1)
    k_rows = k1a[srcs].reshape(NCORES, ELOC, F1)
    v_rows = v1a[srcs].reshape(NCORES, ELOC, F1)
    ea_bf = ea_g.astype(bf)

    kvkea = np.zeros((NCORES, ELOC, H1 * KW), dtype=bf)
    eav = np.zeros((NCORES, ELOC, H1 * EAVW), dtype=bf)
    for h in range(H1):
        kvkea[:, :, h * KW : h * KW + HID] = k_rows[:, :, h * HID : (h + 1) * HID]
        kvkea[:, :, h * KW + HID : (h + 1) * KW] = ea_bf
        eav[:, :, h * EAVW] = evalid.astype(bf)
        eav[:, :, h * EAVW + 1 : h * EAVW + 33] = ea_bf
        eav[:, :, h * EAVW + 33 : (h + 1) * EAVW] = \
            v_rows[:, :, h * HID : (h + 1) * HID]
    kvkea_p = _pack_edge_rows(kvkea)
    eav_p = _pack_edge_rows(eav)
    # interleave q|qwe per head to match the kvkea [k_h|ea] layout
    qaug_i = np.zeros((NTOT, QAW), dtype=bf)
    for h in range(H1):
        qaug_i[:, h * KW : h * KW + HID] = qaug1[:, h * HID : (h + 1) * HID]
        qaug_i[:, h * KW + HID : (h + 1) * KW] = \
            qaug1[:, F1 + h * EDGE_DIM : F1 + (h + 1) * EDGE_DIM]
    qaug_p = _pack_node_cols(qaug_i.reshape(NCORES, NLOC, QAW))
    skip_p = _pack_node_cols(skip1.reshape(NCORES, NLOC, F1))

    # l1 consts: W2cat [512, 288] = [Wk2^T|Wv2^T|Wq2^T|Wqe2|Ws2^T]
    Wqe2 = W["Wq2"].T @ W["We2"]  # [512, 32]
    bqe2 = W["bq2"] @ W["We2"]    # [32]
    W2cat = np.concatenate(
        [W["Wk2"].T, W["Wv2"].T, W["Wq2"].T, Wqe2, W["Ws2"].T], axis=1)
    b2row = np.concatenate(
        [np.zeros(OUT, np.float32), W["bv2"], W["bq2"], bqe2, W["bs2"]])
    w2_p = np.ascontiguousarray(
        W2cat.reshape(H1, BLKN, O2W).transpose(1, 0, 2).reshape(BLKN, -1))

    if "l1" not in _built:
        _built["l1"] = _build_l1()
    in_maps1 = []
    for c in range(NCORES):
        in_maps1.append({
            "kvkea": kvkea_p[c], "eav": eav_p[c],
            "S": S_p[c], "ST": ST_p[c],
            "qaug": qaug_p[c], "skip": skip_p[c],
            "we": np.concatenate(
                [W["bv1"][None, :], W["We1"].T], axis=0).astype(bf),
            "w2": w2_p.astype(bf),
            "b2": b2row[None, :].astype(bf),
            "ones": ones, "ident": ident,
        })
    r1 = _run(_built["l1"], in_maps1)
    out2 = np.concatenate([r1.results[c]["out2"] for c in range(NCORES)], axis=0)
    k2a, v2a = out2[:, :OUT], out2[:, OUT : 2 * OUT]
    qaug2 = out2[:, 2 * OUT : 2 * OUT + QAW2]   # q2|qwe2 (96)
    s2a = out2[:, 2 * OUT + QAW2 :]

    k2_rows = k2a[srcs].reshape(NCORES, ELOC, OUT)
    v2_rows = v2a[srcs].reshape(NCORES, ELOC, OUT)
    kvkea2 = np.zeros((NCORES, ELOC, KW2), dtype=bf)
    kvkea2[:, :, :OUT] = k2_rows
    kvkea2[:, :, OUT:] = ea_bf
    eav2 = np.zeros((NCORES, ELOC, EAVW2), dtype=bf)
    eav2[:, :, 0] = evalid.astype(bf)
    eav2[:, :, 1:33] = ea_bf
    eav2[:, :, 33:] = v2_rows
    kvkea2_p = _pack_edge_rows(kvkea2)
    eav2_p = _pack_edge_rows(eav2)
    qaug2_p = _pack_node_cols(qaug2.reshape(NCORES, NLOC, QAW2))
    s2_p = _pack_node_cols(s2a.reshape(NCORES, NLOC, OUT))

    if "l2" not in _built:
        _built["l2"] = _build_l2()
    in_maps2 = []
    for c in range(NCORES):
        in_maps2.append({
            "kvkea2": kvkea2_p[c], "eav2": eav2_p[c],
            "S": S_p[c], "ST": ST_p[c],
            "qaug2": qaug2_p[c], "s2": s2_p[c],
            "we2": np.ascontiguousarray(W["We2"].T).astype(bf),
            "ident": ident,
        })
    r2 = _run(_built["l2"], in_maps2)
    z_all = np.concatenate([r2.results[c]["z"] for c in range(NCORES)], axis=0)

    zout = z_all[plan.node_gslot]
    if PROFILE:
        LAST_EXEC_NS = sum(int(t) for t in _exec_ns if t) if all(_exec_ns) else None
        LAST_TRACES = _traces
    return zout.astype(np.float32)


# revision 4
# speedup vs baseline: 1.0041x; 1.0002x over previous
"""TransformerConv 2-layer GNN encoder on 8 Trainium2 NeuronCores, v2.

Strategy (dst-sharded graph parallelism, refactored for engine balance):
  - Same node/edge binning as v1 (degree-balanced FFD: 8 cores x 20 blocks x
    128 slots, 8 edge-chunks of 128 per block).
  - Bias algebra: k-bias cancels in segment softmax; v-bias folds into the
    skip bias. So kv tables carry no bias.
  - alpha = q.k[src] + q.(ea@WeT) = q.k[src] + (q@We).ea, with Wqe = Wq^T@We
    fused on the host (weights-only preprocessing). Launch 0 emits per-node
    [k | v | q | qwe | skip]; host gathers [k_h|ea] per edge so alpha is one
    fused DVE tensor_tensor_reduce per head.
  - Value-side edge term: scatter exp-weighted raw ea (32 cols) with the same
    S matmul, multiply by We once per block after aggregation.
  - exp-weighting via DVE tensor_scalar with per-partition scalar (4x mode).
  - All engine work split across PE / ACT / DVE / GPSIMD.
"""

import sys

sys.path.insert(0, "/opt/trn_rl_repo")

import json

import numpy as np

# ----------------------------------------------------------------------------
# Problem constants (hardcoded per contract)
# ----------------------------------------------------------------------------
N, E, IN_DIM, EDGE_DIM, HID, OUT = 20000, 160000, 128, 32, 128, 64
H1 = 4
F1 = H1 * HID  # 512
NCORES = 8
BLKS = 20          # dst blocks per core
BLKN = 128         # nodes per block
NLOC = BLKS * BLKN  # 2560 nodes per core
NTOT = NCORES * NLOC  # 20480 slots
CPB = 8            # chunks per block
T = 128            # edges per chunk
ECHUNKS = BLKS * CPB  # 160 chunks per core
ELOC = ECHUNKS * T    # 20480 edge slots per core

ISQ1 = 1.0 / np.sqrt(np.float32(HID))
ISQ2 = 1.0 / np.sqrt(np.float32(OUT))
DENOM_EPS = 1e-30

# layout widths
KW = 160           # [k_h(128) | ea(32)] per head in kvkea pack
EAVW = 161         # [1 | ea(32) | v_h(128)] per head in eav pack
L0W = 2176         # l0 out: k(512) v(512) q(512) qwe(128) skip(512)
QAW = F1 + H1 * EDGE_DIM  # 640: q(512) | qwe(128)
O2W = 288          # l1 out: k2(64) v2(64) q2(64) qwe2(32) s2(64)
KW2 = OUT + EDGE_DIM   # 96
EAVW2 = 1 + EDGE_DIM + OUT  # 97
QAW2 = OUT + EDGE_DIM  # 96

# ----------------------------------------------------------------------------
# Walrus single-wait shim + NTFF profiling hook (inlined; must be
# self-contained).
# ----------------------------------------------------------------------------
_shim_installed = False


def _split_waits_in_bir(bir_bytes: bytes) -> bytes:
    d = json.loads(bir_bytes)
    for fn in d.get("functions", []):
        for blk in fn.get("blocks", []):
            new_insts = []
            for ins in blk.get("instructions", []):
                si = ins.get("sync_info") or {}
                waits = si.get("on_wait") or []
                if len(waits) > 1:
                    for k, w in enumerate(waits[:-1]):
                        ev = {
                            "name": f"{ins['name']}_wsplit{k}",
                            "opcode": "EventSemaphore",
                            "engine": ins["engine"],
                            "ins": [],
                            "outs": [],
                            "sync_info": {"on_wait": [w], "on_update": []},
                        }
                        if "debug" in ins:
                            ev["debug"] = ins["debug"]
                        new_insts.append(ev)
                    si["on_wait"] = [waits[-1]]
                new_insts.append(ins)
            blk["instructions"] = new_insts
    return json.dumps(d).encode()


def _install_shim():
    global _shim_installed
    if _shim_installed:
        return
    import concourse.bass2jax as bass2jax
    import concourse.bass_utils as bass_utils

    orig = bass_utils.compile_bir_kernel

    def wrapped(bir_json, tmpdir, neff_name="file.neff"):
        if isinstance(bir_json, str):
            bir_json = bir_json.encode()
        return orig(_split_waits_in_bir(bir_json), tmpdir, neff_name=neff_name)

    bass_utils.compile_bir_kernel = wrapped
    bass2jax.compile_bir_kernel = wrapped

    import types

    try:
        from antenv import axon_hooks  # noqa: F401
    except ImportError:
        import antenv

        mod = types.ModuleType("antenv.axon_hooks")
        _state = {"hook": None}
        mod.set_axon_ntff_profile_hook = lambda h: _state.__setitem__("hook", h)
        mod.get_axon_ntff_profile_hook = lambda: _state["hook"]
        sys.modules["antenv.axon_hooks"] = mod
        antenv.axon_hooks = mod
        try:
            from trn_agent_boot.trn_boot import _ntff_profile_via_ctypes

            hook = _ntff_profile_via_ctypes("/opt/axon/libaxon_pjrt.so")
            if hook is not None:
                mod.set_axon_ntff_profile_hook(hook)
        except Exception:
            pass
    _shim_installed = True


# ----------------------------------------------------------------------------
# Host-side graph planning (identical to v1)
# ----------------------------------------------------------------------------
class _Plan:
    pass


def _make_plan(ei: np.ndarray) -> _Plan:
    src = np.asarray(ei[0], dtype=np.int64)
    dst = np.asarray(ei[1], dtype=np.int64)
    deg = np.bincount(dst, minlength=N)

    nbins = NCORES * BLKS
    cap_e = CPB * T
    cap_n = BLKN

    order = np.argsort(-deg, kind="stable")
    bin_e = np.zeros(nbins, dtype=np.int64)
    bin_n = np.zeros(nbins, dtype=np.int64)
    node_bin = np.empty(N, dtype=np.int64)
    start = 0
    for nd in order:
        d = deg[nd]
        placed = False
        for k in range(nbins):
            b = (start + k) % nbins
            if bin_e[b] + d <= cap_e and bin_n[b] < cap_n:
                node_bin[nd] = b
                bin_e[b] += d
                bin_n[b] += 1
                start = (b + 1) % nbins
                placed = True
                break
        if not placed:
            raise RuntimeError("bin packing failed")

    node_slot = np.empty(N, dtype=np.int64)
    fill = np.zeros(nbins, dtype=np.int64)
    for nd in range(N):
        b = node_bin[nd]
        node_slot[nd] = fill[b]
        fill[b] += 1

    node_gslot = node_bin * BLKN + node_slot

    edge_bin = node_bin[dst]
    eorder = np.argsort(edge_bin, kind="stable")
    sorted_bins = edge_bin[eorder]
    bin_starts = np.searchsorted(sorted_bins, np.arange(nbins))
    bin_ends = np.searchsorted(sorted_bins, np.arange(nbins), side="right")

    edge_src_gslot = np.zeros((NCORES, ELOC), dtype=np.int64)
    edge_id = np.full((NCORES, ELOC), -1, dtype=np.int64)
    edge_dslot = np.full((NCORES, ELOC), -1, dtype=np.int64)
    for b in range(nbins):
        core = b // BLKS
        blk = b % BLKS
        s, e = bin_starts[b], bin_ends[b]
        eids = eorder[s:e]
        ne = len(eids)
        base = blk * cap_e
        edge_id[core, base : base + ne] = eids
        edge_src_gslot[core, base : base + ne] = node_gslot[src[eids]]
        edge_dslot[core, base : base + ne] = node_slot[dst[eids]]

    p = _Plan()
    p.node_gslot = node_gslot
    p.edge_src_gslot = edge_src_gslot
    p.edge_id = edge_id
    p.edge_dslot = edge_dslot
    return p


def _build_S_packed(plan):
    """S_p[core, b, t, c*BLKN + d] (scatter) and ST_p (gather), bf16."""
    import ml_dtypes

    S = np.zeros((NCORES, ECHUNKS, T, BLKN), dtype=np.float32)
    dslot = plan.edge_dslot.reshape(NCORES, ECHUNKS, T)
    c_idx, ch_idx, t_idx = np.nonzero(dslot >= 0)
    S[c_idx, ch_idx, t_idx, dslot[c_idx, ch_idx, t_idx]] = 1.0
    bf = ml_dtypes.bfloat16
    f8 = ml_dtypes.float8_e4m3fn
    S_p = np.ascontiguousarray(
        S.reshape(NCORES, BLKS, CPB, T, BLKN).transpose(0, 1, 3, 2, 4)
        .reshape(NCORES, BLKS, T, CPB * BLKN)
    ).astype(f8)
    ST_p = np.ascontiguousarray(
        S.reshape(NCORES, BLKS, CPB, T, BLKN).transpose(0, 1, 4, 2, 3)
        .reshape(NCORES, BLKS, BLKN, CPB * T)
    ).astype(f8)
    return S_p, ST_p


def _pack_edge_rows(rows):
    """[NCORES, ELOC, W] -> [NCORES, BLKS, T, CPB*W] block-chunk-major."""
    W = rows.shape[-1]
    return np.ascontiguousarray(
        rows.reshape(NCORES, BLKS, CPB, T, W).transpose(0, 1, 3, 2, 4)
        .reshape(NCORES, BLKS, T, CPB * W)
    )


def _pack_node_cols(arr):
    """[NCORES, NLOC, W] -> [NCORES, BLKN, BLKS*W] (partition-major preload)."""
    W = arr.shape[-1]
    return np.ascontiguousarray(
        arr.reshape(NCORES, BLKS, BLKN, W).transpose(0, 2, 1, 3)
        .reshape(NCORES, BLKN, BLKS * W)
    )


# ----------------------------------------------------------------------------
# Bass kernel builders
# ----------------------------------------------------------------------------
_built = {}


def _get_nc():
    import concourse.bass as bass

    return bass.Bass(target_bir_lowering=False, trn_type="TRN2")


def _build_l0():
    """Transposed node-phase layer1: out0T = (x @ [Wk|Wv|Wq|Wqe|Ws])^T.

    Weight column-chunks are the stationary operand so PE streams node
    columns continuously; biases are per-partition (ACT Identity bias AP).
    k/v chunks (no bias) evacuate on DVE, the rest on ACT.
    """
    import concourse.mybir as mybir
    from concourse.tile import TileContext

    dt = mybir.dt
    bf, f32 = dt.bfloat16, dt.float32
    nc = _get_nc()
    NCH = L0W // 128  # 17 weight chunks
    NSL = NLOC // 512  # 5 node column slices
    xT = nc.dram_tensor("xT", [IN_DIM, NLOC], bf, kind="ExternalInput")
    W0 = nc.dram_tensor("W0", [IN_DIM, L0W], bf, kind="ExternalInput")
    b0c = nc.dram_tensor("b0c", [128, NCH], f32, kind="ExternalInput")
    out0T = nc.dram_tensor("out0T", [L0W, NLOC], bf, kind="ExternalOutput")

    AF = mybir.ActivationFunctionType

    with TileContext(nc) as tc:
        with (
            tc.tile_pool(name="const", bufs=1) as cpool,
            tc.tile_pool(name="sb", bufs=4) as pool,
            tc.tile_pool(name="ps", bufs=6, space="PSUM") as psp,
        ):
            w = cpool.tile([IN_DIM, L0W], bf)
            nc.sync.dma_start(w[:], W0[:])
            bt = cpool.tile([128, NCH], f32)
            nc.sync.dma_start(bt[:], b0c[:])
            xfull = cpool.tile([IN_DIM, NLOC], bf)
            nc.sync.dma_start(xfull[:], xT[:])

            for j in range(NCH):
                for n in range(NSL):
                    ps = psp.tile([128, 512], f32, tag="ps")
                    nc.tensor.matmul(
                        ps[:], w[:, j * 128 : (j + 1) * 128],
                        xfull[:, n * 512 : (n + 1) * 512],
                        start=True, stop=True,
                    )
                    res = pool.tile([128, 512], bf, tag="res")
                    if j < 8:  # k|v chunks: no bias
                        nc.vector.tensor_copy(res[:], ps[:])
                    else:
                        nc.scalar.activation(res[:], ps[:], AF.Identity,
                                             bias=bt[:, j : j + 1])
                    nc.sync.dma_start(
                        out0T[j * 128 : (j + 1) * 128,
                              n * 512 : (n + 1) * 512], res[:])
    return nc


def _build_l1():
    """Layer-1 edge phase + fused layer-2 node phase."""
    import concourse.mybir as mybir
    from concourse.tile import TileContext

    dt = mybir.dt
    f32, bf = dt.float32, dt.bfloat16
    nc = _get_nc()

    kvkead = nc.dram_tensor("kvkea", [BLKS, T, CPB * H1 * KW], bf, kind="ExternalInput")
    eavd = nc.dram_tensor("eav", [BLKS, T, CPB * H1 * EAVW], bf, kind="ExternalInput")
    f8 = dt.float8e4
    Sd = nc.dram_tensor("S", [BLKS, T, CPB * BLKN], f8, kind="ExternalInput")
    STd = nc.dram_tensor("ST", [BLKS, BLKN, CPB * T], f8, kind="ExternalInput")
    qaugd = nc.dram_tensor("qaug", [BLKN, BLKS * QAW], bf, kind="ExternalInput")
    skipd = nc.dram_tensor("skip", [BLKN, BLKS * F1], bf, kind="ExternalInput")
    wed = nc.dram_tensor("we", [EDGE_DIM + 1, F1], bf, kind="ExternalInput")
    w2d = nc.dram_tensor("w2", [BLKN, H1 * O2W], bf, kind="ExternalInput")
    b2d = nc.dram_tensor("b2", [1, O2W], bf, kind="ExternalInput")
    onesd = nc.dram_tensor("ones", [1, BLKN], bf, kind="ExternalInput")
    identd = nc.dram_tensor("ident", [BLKN, BLKN], bf, kind="ExternalInput")
    out2 = nc.dram_tensor("out2", [NLOC, O2W], bf, kind="ExternalOutput")

    AF = mybir.ActivationFunctionType
    ALU = mybir.AluOpType

    with TileContext(nc) as tc:
        with (
            tc.tile_pool(name="const", bufs=1) as cpool,
            tc.tile_pool(name="blk", bufs=3) as bpool,
            tc.tile_pool(name="ck", bufs=6) as kpool,
            tc.tile_pool(name="ep", bufs=2) as epool,
            tc.tile_pool(name="psqt", bufs=2, space="PSUM") as psq_p,
            tc.tile_pool(name="psacc", bufs=2, space="PSUM") as psa_p,
            tc.tile_pool(name="psdea", bufs=1, space="PSUM") as psd_p,
            tc.tile_pool(name="pstail", bufs=1, space="PSUM") as pst_p,
        ):
            we = cpool.tile([EDGE_DIM + 1, F1], bf)
            nc.sync.dma_start(we[:], wed[:])
            w2 = cpool.tile([BLKN, H1 * O2W], bf)
            nc.sync.dma_start(w2[:], w2d[:])
            b2 = cpool.tile([1, O2W], bf)
            nc.sync.dma_start(b2[:], b2d[:])
            on = cpool.tile([1, BLKN], bf)
            nc.sync.dma_start(on[:], onesd[:])
            ident = cpool.tile([BLKN, BLKN], bf)
            nc.sync.dma_start(ident[:], identd[:])
            qaug_all = cpool.tile([BLKN, BLKS * QAW], bf)
            nc.sync.dma_start(qaug_all[:], qaugd[:])
            skip_all = cpool.tile([BLKN, BLKS * F1], bf)
            nc.sync.dma_start(skip_all[:], skipd[:])

            for b in range(BLKS):
                kvkea = bpool.tile([T, CPB * H1 * KW], bf, tag="kvkea")
                nc.sync.dma_start(kvkea[:], kvkead[b])
                eav = bpool.tile([T, CPB * H1 * EAVW], bf, tag="eav")
                nc.sync.dma_start(eav[:], eavd[b])
                sb_ = bpool.tile([T, CPB * BLKN], f8, tag="sb_")
                nc.sync.dma_start(sb_[:], Sd[b])
                stb = bpool.tile([BLKN, CPB * T], f8, tag="stb")
                nc.sync.dma_start(stb[:], STd[b])

                qaug = qaug_all[:, b * QAW : (b + 1) * QAW]
                skipb = skip_all[:, b * F1 : (b + 1) * F1]

                # psnum [BLKN, 512]; psdea [BLKN, 132]: [den(1)|psea(32)] x4
                psnum = psa_p.tile([BLKN, F1], f32, tag="psnum")
                psdea = psd_p.tile([BLKN, H1 * 33], f32, tag="psdea")

                def qt_mm(i):
                    """qt gather [T, 640] in one PSUM tile, halves bank-split.
                    Issued one chunk ahead so PE never stalls on the chain."""
                    st_ = stb[:, i * T : (i + 1) * T]
                    qt_ps = psq_p.tile([T, 1024], f32, tag="qt_ps")
                    nc.tensor.matmul(qt_ps[:, : 2 * KW], st_, qaug[:, : 2 * KW],
                                     start=True, stop=True)
                    nc.tensor.matmul(qt_ps[:, 512 : 512 + 2 * KW], st_,
                                     qaug[:, 2 * KW :], start=True, stop=True)
                    return qt_ps

                qt_cur = qt_mm(0)
                for i in range(CPB):
                    s_ = sb_[:, i * BLKN : (i + 1) * BLKN]

                    qts = kpool.tile([T, H1 * KW], bf, tag="qts")
                    nc.scalar.activation(
                        qts[:],
                        qt_cur[:].rearrange("p (g w) -> p g w", g=2)[:, :, : 2 * KW],
                        AF.Copy,
                    )
                    if i + 1 < CPB:
                        qt_cur = qt_mm(i + 1)

                    # alpha per head: q.k + qwe.ea over the packed 160 cols
                    kv_ = kvkea[:, i * H1 * KW : (i + 1) * H1 * KW]
                    prod = kpool.tile([T, H1 * KW], bf, tag="prod")
                    nc.vector.tensor_tensor(prod[:], qts[:], kv_, ALU.mult)
                    alpha = kpool.tile([T, H1], f32, tag="alpha")
                    nc.vector.tensor_reduce(
                        alpha[:],
                        prod[:].rearrange("p (h w) -> p h w", h=H1),
                        mybir.AxisListType.X, ALU.add,
                    )
                    exb = kpool.tile([T, H1], f32, tag="exb")
                    nc.scalar.activation(exb[:], alpha[:], AF.Exp, scale=ISQ1)

                    # exp-weighted [1|ea|v_h] per head (one broadcast mult)
                    eav_ = eav[:, i * H1 * EAVW : (i + 1) * H1 * EAVW]
                    exw = kpool.tile([T, H1 * EAVW], bf, tag="exw")
                    exbc = (
                        exb[:].rearrange("p (h o) -> p h o", h=H1)
                        .broadcast_to([T, H1, EAVW])
                    )
                    nc.gpsimd.tensor_tensor(
                        exw[:].rearrange("p (h w) -> p h w", h=H1),
                        eav_.rearrange("p (h w) -> p h w", h=H1),
                        exbc, ALU.mult,
                    )

                    # scatter: num (v part) and den|ea part
                    exw_v = exw[:].rearrange("p (h w) -> p h w", h=H1)[:, :, 33:]
                    exw_de = exw[:].rearrange("p (h w) -> p h w", h=H1)[:, :, :33]
                    nc.tensor.matmul(psnum[:], s_, exw_v,
                                     start=(i == 0), stop=False)
                    nc.tensor.matmul(psdea[:], s_, exw_de,
                                     start=(i == 0), stop=(i == CPB - 1))

                # ---- block epilogue ----
                dea_v = psdea[:].rearrange("p (h w) -> p h w", h=H1)
                den = epool.tile([BLKN, H1], f32, tag="den")
                nc.vector.tensor_scalar_max(den[:], dea_v[:, :, 0], DENOM_EPS)
                rcp = epool.tile([BLKN, H1], f32, tag="rcp")
                nc.vector.reciprocal(rcp[:], den[:])

                # fold value-side edge term: psnum_h += psea_h @ We_h^T
                # evac [den|ea-agg] whole (den row folds the v-bias via
                # we_aug's first row: psnum_h += den_h (x) bv_h + psea_h @ We_h)
                psea_sb = epool.tile([BLKN, H1 * 33], bf, tag="psea_sb")
                nc.scalar.activation(psea_sb[:], psdea[:], AF.Copy)
                pst = pst_p.tile([BLKN, F1], bf, tag="pst")
                for h in range(H1):
                    nc.tensor.transpose(
                        pst[: 33, h * BLKN : (h + 1) * BLKN],
                        psea_sb[:, h * 33 : (h + 1) * 33], ident[:],
                    )
                pseaT = epool.tile([33, H1 * BLKN], bf, tag="pseaT")
                nc.scalar.activation(pseaT[:], pst[:33, :], AF.Copy)
                for h in range(H1):
                    nc.tensor.matmul(
                        psnum[:, h * HID : (h + 1) * HID],
                        pseaT[:, h * BLKN : (h + 1) * BLKN],
                        we[:, h * HID : (h + 1) * HID],
                        start=False, stop=(h == H1 - 1),
                    )

                # h = relu(psnum*rcp + skip)
                attn = epool.tile([BLKN, F1], bf, tag="attn")
                for h in range(H1):
                    nc.scalar.activation(
                        attn[:, h * HID : (h + 1) * HID],
                        psnum[:, h * HID : (h + 1) * HID],
                        AF.Copy, scale=rcp[:, h : h + 1],
                    )
                hpre = epool.tile([BLKN, F1], bf, tag="hpre")
                nc.gpsimd.tensor_tensor(hpre[:], attn[:], skipb, ALU.add)
                hrelu = epool.tile([BLKN, F1], bf, tag="hrelu")
                nc.vector.tensor_scalar_max(hrelu[:], hpre[:], 0.0)

                # transpose h, then layer-2 node matmuls (reuses the pst
                # buffer; Tile serializes on the pseaT copy above)
                pst2 = pst_p.tile([BLKN, F1], bf, tag="pst")
                for fb in range(H1):
                    sl = slice(fb * BLKN, (fb + 1) * BLKN)
                    nc.tensor.transpose(pst2[:, sl], hrelu[:, sl], ident[:])
                hT = epool.tile([BLKN, F1], bf, tag="hT")
                nc.scalar.activation(hT[:, : 2 * BLKN], pst2[:, : 2 * BLKN], AF.Copy)
                nc.vector.tensor_copy(hT[:, 2 * BLKN :], pst2[:, 2 * BLKN :])

                # ps2 reuses the psnum tag's other buffer (freed after the
                # previous block's attn reads)
                ps2_t = psa_p.tile([BLKN, F1], f32, tag="psnum")
                ps2 = ps2_t[:, :O2W]
                for fb in range(H1):
                    nc.tensor.matmul(
                        ps2, hT[:, fb * BLKN : (fb + 1) * BLKN],
                        w2[:, fb * O2W : (fb + 1) * O2W],
                        start=(fb == 0), stop=False,
                    )
                nc.tensor.matmul(ps2, on[:], b2[:], start=False, stop=True)
                o2 = epool.tile([BLKN, O2W], bf, tag="o2")
                nc.scalar.activation(o2[:], ps2, AF.Copy)
                nc.sync.dma_start(out2[b * BLKN : (b + 1) * BLKN, :], o2[:])
    return nc


def _build_l2():
    """Layer-2 edge phase: z = attn2 + s2 (single head, C=64)."""
    import concourse.mybir as mybir
    from concourse.tile import TileContext

    dt = mybir.dt
    f32, bf = dt.float32, dt.bfloat16
    nc = _get_nc()

    kvkead = nc.dram_tensor("kvkea2", [BLKS, T, CPB * KW2], bf, kind="ExternalInput")
    eavd = nc.dram_tensor("eav2", [BLKS, T, CPB * EAVW2], bf, kind="ExternalInput")
    f8 = dt.float8e4
    Sd = nc.dram_tensor("S", [BLKS, T, CPB * BLKN], f8, kind="ExternalInput")
    STd = nc.dram_tensor("ST", [BLKS, BLKN, CPB * T], f8, kind="ExternalInput")
    qaugd = nc.dram_tensor("qaug2", [BLKN, BLKS * QAW2], bf, kind="ExternalInput")
    s2d = nc.dram_tensor("s2", [BLKN, BLKS * OUT], bf, kind="ExternalInput")
    wed = nc.dram_tensor("we2", [EDGE_DIM, OUT], bf, kind="ExternalInput")
    identd = nc.dram_tensor("ident", [BLKN, BLKN], bf, kind="ExternalInput")
    z = nc.dram_tensor("z", [NLOC, OUT], f32, kind="ExternalOutput")

    AF = mybir.ActivationFunctionType
    ALU = mybir.AluOpType

    with TileContext(nc) as tc:
        with (
            tc.tile_pool(name="const", bufs=1) as cpool,
            tc.tile_pool(name="blk", bufs=3) as bpool,
            tc.tile_pool(name="ck", bufs=6) as kpool,
            tc.tile_pool(name="ep", bufs=2) as epool,
            tc.tile_pool(name="psqt", bufs=2, space="PSUM") as psq_p,
            tc.tile_pool(name="psacc", bufs=2, space="PSUM") as psa_p,
            tc.tile_pool(name="pstail", bufs=2, space="PSUM") as pst_p,
        ):
            we = cpool.tile([EDGE_DIM, OUT], bf)
            nc.sync.dma_start(we[:], wed[:])
            ident = cpool.tile([BLKN, BLKN], bf)
            nc.sync.dma_start(ident[:], identd[:])
            qaug_all = cpool.tile([BLKN, BLKS * QAW2], bf)
            nc.sync.dma_start(qaug_all[:], qaugd[:])
            s2_all = cpool.tile([BLKN, BLKS * OUT], bf)
            nc.sync.dma_start(s2_all[:], s2d[:])

            for b in range(BLKS):
                kvkea = bpool.tile([T, CPB * KW2], bf, tag="kvkea")
                nc.sync.dma_start(kvkea[:], kvkead[b])
                eav = bpool.tile([T, CPB * EAVW2], bf, tag="eav")
                nc.sync.dma_start(eav[:], eavd[b])
                sb_ = bpool.tile([T, CPB * BLKN], f8, tag="sb_")
                nc.sync.dma_start(sb_[:], Sd[b])
                stb = bpool.tile([BLKN, CPB * T], f8, tag="stb")
                nc.sync.dma_start(stb[:], STd[b])

                qaug = qaug_all[:, b * QAW2 : (b + 1) * QAW2]
                s2b = s2_all[:, b * OUT : (b + 1) * OUT]

                psaug = psa_p.tile([BLKN, EAVW2], f32, tag="psaug")

                def qt_mm2(i):
                    st_ = stb[:, i * T : (i + 1) * T]
                    qt_ps = psq_p.tile([T, QAW2], f32, tag="qt_ps")
                    nc.tensor.matmul(qt_ps[:], st_, qaug, start=True, stop=True)
                    return qt_ps

                qt_cur = qt_mm2(0)
                for i in range(CPB):
                    s_ = sb_[:, i * BLKN : (i + 1) * BLKN]

                    qts = kpool.tile([T, QAW2], bf, tag="qts")
                    nc.scalar.activation(qts[:], qt_cur[:], AF.Copy)
                    if i + 1 < CPB:
                        qt_cur = qt_mm2(i + 1)

                    kv_ = kvkea[:, i * KW2 : (i + 1) * KW2]
                    prod = kpool.tile([T, KW2], bf, tag="prod")
                    nc.vector.tensor_tensor(prod[:], qts[:], kv_, ALU.mult)
                    alpha = kpool.tile([T, 1], f32, tag="alpha")
                    nc.vector.tensor_reduce(
                        alpha[:], prod[:], mybir.AxisListType.X, ALU.add,
                    )
                    exb = kpool.tile([T, 1], f32, tag="exb")
                    nc.scalar.activation(exb[:], alpha[:], AF.Exp, scale=ISQ2)

                    eav_ = eav[:, i * EAVW2 : (i + 1) * EAVW2]
                    exw = kpool.tile([T, EAVW2], bf, tag="exw")
                    exbc = (
                        exb[:].rearrange("p (h o) -> p h o", h=1)
                        .broadcast_to([T, 1, EAVW2])
                    )
                    nc.gpsimd.tensor_tensor(
                        exw[:].rearrange("p (h w) -> p h w", h=1),
                        eav_.rearrange("p (h w) -> p h w", h=1),
                        exbc, ALU.mult,
                    )

                    nc.tensor.matmul(psaug[:], s_, exw[:],
                                     start=(i == 0), stop=(i == CPB - 1))

                # ---- block epilogue ----
                den = epool.tile([BLKN, 1], f32, tag="den")
                nc.vector.tensor_scalar_max(den[:], psaug[:, 0:1], DENOM_EPS)
                rcp = epool.tile([BLKN, 1], f32, tag="rcp")
                nc.vector.reciprocal(rcp[:], den[:])

                psea_sb = epool.tile([BLKN, EDGE_DIM], bf, tag="psea_sb")
                nc.scalar.activation(psea_sb[:], psaug[:, 1:33], AF.Copy)
                pst = pst_p.tile([EDGE_DIM, BLKN], bf, tag="pst")
                nc.tensor.transpose(pst[:], psea_sb[:], ident[:])
                pseaT = epool.tile([EDGE_DIM, BLKN], bf, tag="pseaT")
                nc.vector.tensor_copy(pseaT[:], pst[:])
                nc.tensor.matmul(psaug[:, 33:], pseaT[:], we[:],
                                 start=False, stop=True, skip_group_check=True)

                attn = epool.tile([BLKN, OUT], f32, tag="attn")
                nc.scalar.activation(attn[:], psaug[:, 33:], AF.Copy,
                                     scale=rcp[:])
                zb = epool.tile([BLKN, OUT], f32, tag="zb")
                nc.gpsimd.tensor_tensor(zb[:], attn[:], s2b, ALU.add)
                nc.sync.dma_start(z[b * BLKN : (b + 1) * BLKN, :], zb[:])
    return nc


# ----------------------------------------------------------------------------
# Kernel entry point
# ----------------------------------------------------------------------------
PROFILE = False
LAST_EXEC_NS = None
LAST_TRACES = None


def kernel(**inputs):
    global LAST_EXEC_NS, LAST_TRACES
    _install_shim()
    import ml_dtypes

    from concourse import bass_utils

    bf = ml_dtypes.bfloat16

    def _run(nc, in_maps):
        r = bass_utils.run_bass_kernel_spmd(
            nc, in_maps, core_ids=list(range(NCORES)), trace=PROFILE
        )
        if PROFILE:
            _exec_ns.append(r.exec_time_ns)
            _traces.append(r.instructions_and_trace)
        return r

    _exec_ns, _traces = [], []

    x = np.asarray(inputs["x"], dtype=np.float32)
    ei = np.asarray(inputs["ei"])
    ea = np.asarray(inputs["ea"], dtype=np.float32)
    W = {k: np.asarray(v, dtype=np.float32) for k, v in inputs.items()
         if k not in ("x", "ei", "ea")}

    plan = _make_plan(ei)
    S_p, ST_p = _build_S_packed(plan)

    # gathered edge attrs [NCORES, ELOC, EDGE_DIM] (0 for pads)
    eid = plan.edge_id
    evalid = eid >= 0
    ea_g = np.zeros((NCORES, ELOC, EDGE_DIM), dtype=np.float32)
    ea_g[evalid] = ea[eid[evalid]]

    # node features in slot order
    x_slots = np.zeros((NTOT, IN_DIM), dtype=np.float32)
    x_slots[plan.node_gslot] = x
    xT_all = np.ascontiguousarray(x_slots.T).astype(bf)

    ones = np.ones((1, BLKN), dtype=np.float32).astype(bf)
    ident = np.eye(BLKN, dtype=np.float32).astype(bf)

    # ---------------- launch 0 ----------------
    # fused weights: Wqe1[h] = Wq1_h^T @ We1_h  [IN, 32]
    Wq1, We1 = W["Wq1"], W["We1"]
    Wqe1 = np.concatenate(
        [Wq1[h * HID : (h + 1) * HID].T @ We1[h * HID : (h + 1) * HID]
         for h in range(H1)], axis=1)  # [IN, 128]
    bqe1 = np.concatenate(
        [W["bq1"][h * HID : (h + 1) * HID] @ We1[h * HID : (h + 1) * HID]
         for h in range(H1)])  # [128]
    W0 = np.concatenate(
        [W["Wk1"].T, W["Wv1"].T, W["Wq1"].T, Wqe1, W["Ws1"].T], axis=1)
    bias_full = np.concatenate(
        [np.zeros(2 * F1, np.float32), W["bq1"], bqe1, W["bs1"]])
    b0c = np.ascontiguousarray(bias_full.reshape(L0W // 128, 128).T)

    if "l0" not in _built:
        _built["l0"] = _build_l0()
    in_maps0 = []
    for c in range(NCORES):
        in_maps0.append({
            "xT": np.ascontiguousarray(xT_all[:, c * NLOC : (c + 1) * NLOC]),
            "W0": W0.astype(bf),
            "b0c": b0c.astype(np.float32),
        })
    r0 = _run(_built["l0"], in_maps0)
    out0 = np.concatenate(
        [np.asarray(r0.results[c]["out0T"]).T for c in range(NCORES)], axis=0)
    k1a, v1a = out0[:, :512], out0[:, 512:1024]
    qaug1 = out0[:, 1024:1664]          # [NTOT, 640] q|qwe
    skip1 = out0[:, 1664:2176]

    # host gathers (pure data movement)
    srcs = plan.edge_src_gslot.reshape(-1)
    k_rows = k1a[srcs].reshape(NCORES, ELOC, F1)
    v_rows = v1a[srcs].reshape(NCORES, ELOC, F1)
    ea_bf = ea_g.astype(bf)

    kvkea = np.zeros((NCORES, ELOC, H1 * KW), dtype=bf)
    eav = np.zeros((NCORES, ELOC, H1 * EAVW), dtype=bf)
    for h in range(H1):
        kvkea[:, :, h * KW : h * KW + HID] = k_rows[:, :, h * HID : (h + 1) * HID]
        kvkea[:, :, h * KW + HID : (h + 1) * KW] = ea_bf
        eav[:, :, h * EAVW] = evalid.astype(bf)
        eav[:, :, h * EAVW + 1 : h * EAVW + 33] = ea_bf
        eav[:, :, h * EAVW + 33 : (h + 1) * EAVW] = \
            v_rows[:, :, h * HID : (h + 1) * HID]
    kvkea_p = _pack_edge_rows(kvkea)
    eav_p = _pack_edge_rows(eav)
    # interleave q|qwe per head to match the kvkea [k_h|ea] layout
    qaug_i = np.zeros((NTOT, QAW), dtype=bf)
    for h in range(H1):
        qaug_i[:, h * KW : h * KW + HID] = qaug1[:, h * HID : (h + 1) * HID]
        qaug_i[:, h * KW + HID : (h + 1) * KW] = \
            qaug1[:, F1 + h * EDGE_DIM : F1 + (h + 1) * EDGE_DIM]
    qaug_p = _pack_node_cols(qaug_i.reshape(NCORES, NLOC, QAW))
    skip_p = _pack_node_cols(skip1.reshape(NCORES, NLOC, F1))

    # l1 consts: W2cat [512, 288] = [Wk2^T|Wv2^T|Wq2^T|Wqe2|Ws2^T]
    Wqe2 = W["Wq2"].T @ W["We2"]  # [512, 32]
    bqe2 = W["bq2"] @ W["We2"]    # [32]
    W2cat = np.concatenate(
        [W["Wk2"].T, W["Wv2"].T, W["Wq2"].T, Wqe2, W["Ws2"].T], axis=1)
    b2row = np.concatenate(
        [np.zeros(OUT, np.float32), W["bv2"], W["bq2"], bqe2, W["bs2"]])
    w2_p = np.ascontiguousarray(
        W2cat.reshape(H1, BLKN, O2W).transpose(1, 0, 2).reshape(BLKN, -1))

    if "l1" not in _built:
        _built["l1"] = _build_l1()
    in_maps1 = []
    for c in range(NCORES):
        in_maps1.append({
            "kvkea": kvkea_p[c], "eav": eav_p[c],
            "S": S_p[c], "ST": ST_p[c],
            "qaug": qaug_p[c], "skip": skip_p[c],
            "we": np.concatenate(
                [W["bv1"][None, :], W["We1"].T], axis=0).astype(bf),
            "w2": w2_p.astype(bf),
            "b2": b2row[None, :].astype(bf),
            "ones": ones, "ident": ident,
        })
    r1 = _run(_built["l1"], in_maps1)
    out2 = np.concatenate([r1.results[c]["out2"] for c in range(NCORES)], axis=0)
    k2a, v2a = out2[:, :OUT], out2[:, OUT : 2 * OUT]
    qaug2 = out2[:, 2 * OUT : 2 * OUT + QAW2]   # q2|qwe2 (96)
    s2a = out2[:, 2 * OUT + QAW2 :]

    k2_rows = k2a[srcs].reshape(NCORES, ELOC, OUT)
    v2_rows = v2a[srcs].reshape(NCORES, ELOC, OUT)
    kvkea2 = np.zeros((NCORES, ELOC, KW2), dtype=bf)
    kvkea2[:, :, :OUT] = k2_rows
    kvkea2[:, :, OUT:] = ea_bf
    eav2 = np.zeros((NCORES, ELOC, EAVW2), dtype=bf)
    eav2[:, :, 0] = evalid.astype(bf)
    eav2[:, :, 1:33] = ea_bf
    eav2[:, :, 33:] = v2_rows
    kvkea2_p = _pack_edge_rows(kvkea2)
    eav2_p = _pack_edge_rows(eav2)
    qaug2_p = _pack_node_cols(qaug2.reshape(NCORES, NLOC, QAW2))
    s2_p = _pack_node_cols(s2a.reshape(NCORES, NLOC, OUT))

    if "l2" not in _built:
        _built["l2"] = _build_l2()
    in_maps2 = []
    for c in range(NCORES):
        in_maps2.append({
            "kvkea2": kvkea2_p[c], "eav2": eav2_p[c],
            "S": S_p[c], "ST": ST_p[c],
            "qaug2": qaug2_p[c], "s2": s2_p[c],
            "we2": np.ascontiguousarray(W["We2"].T).astype(bf),
            "ident": ident,
        })
    r2 = _run(_built["l2"], in_maps2)
    z_all = np.concatenate([r2.results[c]["z"] for c in range(NCORES)], axis=0)

    zout = z_all[plan.node_gslot]
    if PROFILE:
        LAST_EXEC_NS = sum(int(t) for t in _exec_ns if t) if all(_exec_ns) else None
        LAST_TRACES = _traces
    return zout.astype(np.float32)
